# revision 14
# baseline (speedup 1.0000x reference)
"""Expert-parallel MoE conditional feed-forward for 8 Trainium2 NeuronCores.

Problem: x[16,1024], expert_indices[16,2], gate/down_proj[8,2816,1024],
up_proj[8,1024,2816]. Reference computes, per (token, slot) pair with
e = expert_indices[t, a]:
    out[t,a,:] = (silu(x @ gate_proj[e].T) * (x @ down_proj[e].T)) @ up_proj[e].T

Sharding: core k owns expert k and computes its FFN output for ALL 16
tokens (the compute is negligible; the kernel is weight-streaming bound).
The host then gathers rows per expert_indices. This needs no indices on
device and is load-balanced regardless of routing.

Weights and x are cast to fp16 on the host (harness gate is 2e-2; fp16
end-to-end measures 4.7e-4 while fp8 e4m3 is mantissa-limited at 2.7e-2+
per matrix). 17.3 MB per core streams at the ~420 GB/s per-core fabric
ceiling measured when all 8 cores stream (~42 us).

Timeline anatomy (measured): exec_time = [first GpSimd MEMSET ... last
epilogue NOTIFY]. A ~5.8 us start rendezvous is excluded; a fixed
~8.4 us walrus epilogue (per-semaphore zeroing avalanche) is included.
So the controllable part is first-DMA-issue -> out-DMA-complete.

Key structural choices vs the previous revision:
  * Weight chunk 0's first DMA is issued BEFORE xt/eye so streaming (the
    critical path) starts ~1.3 us earlier.
  * Chunks are processed singly (not in pairs): chunk c's 4 up-matmuls
    are deferred only into chunk c+1's gate/down chain. That keeps the
    end-of-stream backlog to one chunk.
  * Chunks 0, 9, 10 stream as split DMAs; Tile's range-level dependency
    tracking lets consuming matmuls start as soon as their slice lands.
    Chunk 10 additionally uses a per-half column layout and is processed
    as two independent 128-wide half-chunks, so the dependency chain
    behind the very last weight byte is one 8-matmul N=256 chain + one
    silu/mul/transpose + 2 up-matmuls.
  * The final PSUM->SBUF copies run in parallel (jb0 on DVE, jb1 on the
    Scalar engine as an activation-Copy) before a single output DMA.

PE scheduling (array packing via PSUM base partition; q3 unusable per
HW bug): q1 (psum rows 32-47) and q2 (rows 64-79) alternate per chunk
for the gate|down chains; q0 (rows 0-15) carries all up-projection
accumulation into psum_out. The [16,128] fp16 intermediates are
transposed to [128,16] via identity matmuls on the PE, cast to fp16 on
the PSUM->SBUF copy, and fed as stationaries.
"""

import sys

for _p in ("/opt/trn_rl_repo", "/opt/pypackages"):
    if _p not in sys.path:
        sys.path.append(_p)

import numpy as np

NUM_EXPERTS = 8
HIDDEN = 1024
INTER = 2816
T = 16
N_CORES = 8
P = 128
PG = 124                  # gate/down weights use partitions 0..123 only:
                          # SDMA engine 15 (the chronically contended one)
                          # serves partitions {92-95, 124-127}; dropping
                          # 124-127 halves its load so it never limits the
                          # stream. Hidden rows 992..1023 go to a shared
                          # "leftover" tile on partitions 0..95.
CW = 256                  # intermediate chunk width
NCHUNK = INTER // CW      # 11
NFULL = NCHUNK - 1        # chunks 0..9 use the full-chunk layout
HC = HIDDEN // P          # 8 strips per gate/down chain (of PG rows each)
U_OFF = 2 * HC * CW       # 4096: offset of up blocks in packed W
WCOLS = U_OFF + 2 * HIDDEN  # 6144
N_UP = 2 * NFULL + 2      # 22 up-matmuls per 512-col output half

_COMPILED = None
LAST_RESULTS = None
TRACE = False


def _build():
    import concourse.bacc as bacc
    import concourse.bass as bass
    import concourse.tile as tile
    from concourse import mybir

    f32 = mybir.dt.float32
    f16 = mybir.dt.float16
    nc = bacc.Bacc("TRN2", target_bir_lowering=False, debug=False,
                   num_devices=N_CORES)
    xt_d = nc.dram_tensor("xt", [PG, HC * T], f16, kind="ExternalInput")
    xtl_d = nc.dram_tensor("xtl", [96, T], f16, kind="ExternalInput")
    eye_d = nc.dram_tensor("eye", [T, T], f16, kind="ExternalInput")
    lt_d = nc.dram_tensor("lt", [96, 4 * 512], f16, kind="ExternalInput")
    w_d = nc.dram_tensor("w", [NCHUNK, P, WCOLS], f16, kind="ExternalInput")
    out_d = nc.dram_tensor("out", [T, HIDDEN], f32, kind="ExternalOutput")

    with tile.TileContext(nc) as tc:
        with (
            tc.tile_pool(name="xp", bufs=1) as xp,
            tc.tile_pool(name="wp", bufs=1) as wp,
            tc.tile_pool(name="ip", bufs=4) as ip,
            tc.tile_pool(name="itp", bufs=1) as itp,
            tc.tile_pool(name="pg", bufs=3, space=bass.MemorySpace.PSUM) as pgp,
            tc.tile_pool(name="tp", bufs=2, space=bass.MemorySpace.PSUM) as tpp,
            tc.tile_pool(name="po", bufs=1, space=bass.MemorySpace.PSUM) as pop,
            tc.tile_pool(name="op", bufs=1) as op,
        ):
            xt = xp.tile([PG, HC * T], f16)
            xtl = xp.tile([96, T], f16)
            eye = xp.tile([T, T], f16)
            lt = xp.tile([96, 4 * 512], f16)
            wt = [wp.tile([P, WCOLS], f16, name=f"w{c}", tag=f"w{c}")
                  for c in range(NCHUNK)]

            # xt/xtl/eye go via GpSimd (SWDGE) so the Sync engine issues
            # only weight DMAs back-to-back (descriptor generation is
            # ~650 ns per dma_start; keeping Sync weight-only shortens the
            # stream ramp). Gate/down DMAs move partitions 0..123 only;
            # the leftover tile lt carries hidden rows 992..1023 for all
            # chunks and lands early. Chunks 0, 9, 10 are split so
            # consuming matmuls start per-slice (Tile tracks range-level
            # DMA->reader deps).
            nc.gpsimd.dma_start(xt[:], xt_d.ap())
            nc.gpsimd.dma_start(xtl[:], xtl_d.ap())
            nc.gpsimd.dma_start(eye[:], eye_d.ap())
            nc.sync.dma_start(wt[0][0:PG, 0:U_OFF // 2],
                              w_d.ap()[0][0:PG, 0:U_OFF // 2])
            nc.sync.dma_start(lt[:], lt_d.ap())
            nc.sync.dma_start(wt[0][0:PG, U_OFF // 2:U_OFF],
                              w_d.ap()[0][0:PG, U_OFF // 2:U_OFF])
            nc.sync.dma_start(wt[0][:, U_OFF:WCOLS],
                              w_d.ap()[0][:, U_OFF:WCOLS])
            for c in range(1, NFULL):
                nc.sync.dma_start(wt[c][0:PG, 0:U_OFF],
                                  w_d.ap()[c][0:PG, 0:U_OFF])
                nc.sync.dma_start(wt[c][:, U_OFF:WCOLS],
                                  w_d.ap()[c][:, U_OFF:WCOLS])
            # chunk 10 (split layout): half h gd at [h*2048,(h+1)*2048),
            # half h up at [4096+h*1024, 4096+(h+1)*1024)
            nc.sync.dma_start(wt[10][0:PG, 0:2048],
                              w_d.ap()[10][0:PG, 0:2048])
            nc.sync.dma_start(wt[10][:, U_OFF:U_OFF + HIDDEN],
                              w_d.ap()[10][:, U_OFF:U_OFF + HIDDEN])
            nc.sync.dma_start(wt[10][0:PG, 2048:U_OFF],
                              w_d.ap()[10][0:PG, 2048:U_OFF])
            nc.sync.dma_start(wt[10][:, U_OFF + HIDDEN:WCOLS],
                              w_d.ap()[10][:, U_OFF + HIDDEN:WCOLS])

            psum_out = pop.tile([T, HIDDEN], f32)
            itall = itp.tile([P, N_UP * T], f16)
            up_count = [0, 0]    # per-jb position in the accumulation chain
            pending = []         # PE thunks deferred from the previous chunk

            # All PE work downstream of a chunk's silu/mul (the transpose
            # and the two-or-four up-matmuls) is deferred into the NEXT
            # chunk's gate/down chain. The PE is strictly in-order, so
            # emitting a transpose right after its own chunk's chain would
            # stall the PE queue ~1.2 us waiting on ACT/DVE; one chunk
            # later the operands are long ready.
            def make_transpose(inter_slice, kidx):
                def t():
                    tp = tpp.tile([P, T], f32, name="tp")
                    nc.tensor.matmul(tp[:], inter_slice, eye[:])
                    nc.vector.tensor_copy(itall[:, kidx * T:(kidx + 1) * T],
                                          tp[:])
                return t

            def make_up(c, kidx, upbase, jb):
                def u():
                    k = up_count[jb]
                    up_count[jb] += 1
                    nc.tensor.matmul(
                        psum_out[:, jb * 512:(jb + 1) * 512],
                        itall[:, kidx * T:(kidx + 1) * T],
                        wt[c][:, upbase + jb * 512:upbase + (jb + 1) * 512],
                        start=(k == 0), stop=(k == N_UP - 1),
                    )
                return u

            def gd_chain(cols_of_hc, base, todo, m, ltlo, first_pop=0):
                # 8 strips of PG=124 hidden rows from the chunk's own tile,
                # then the 32-row leftover strip (hidden 992..1023) from
                # the shared lt tile at partition offset 32*m.
                pgd = pgp.tile([P, 2 * CW], f32, name="pgd")
                for hc in range(HC):
                    lo, width = cols_of_hc(hc)
                    nc.tensor.matmul(
                        pgd[base:base + T, 0:width],
                        xt[:, hc * T:(hc + 1) * T],
                        wt_cur[0:PG, lo:lo + width],
                        start=(hc == 0), stop=False,
                    )
                    if todo and hc >= first_pop:
                        todo.pop(0)()
                lo, width = cols_of_hc(0)
                nc.tensor.matmul(
                    pgd[base:base + T, 0:width],
                    xtl[32 * m:32 * m + 32, :],
                    lt[32 * m:32 * m + 32, ltlo:ltlo + width],
                    start=False, stop=True,
                )
                while todo:
                    todo.pop(0)()
                return pgd

            def silu_mul(pgd, base, width):
                s1 = ip.tile([T, width], f32, name="s1")
                nc.scalar.activation(s1[:], pgd[base:base + T, 0:width],
                                     mybir.ActivationFunctionType.Silu)
                inter = ip.tile([T, width], f16, name="inter")
                nc.vector.tensor_mul(inter[:], s1[:],
                                     pgd[base:base + T, width:2 * width])
                return inter

            for c in range(NFULL):
                base = 32 if c % 2 == 0 else 64
                wt_cur = wt[c]
                todo, pending = pending, []
                pgd = gd_chain(lambda hc: (hc * 2 * CW, 2 * CW), base, todo,
                               m=c % 3, ltlo=512 * (c // 3))
                inter = silu_mul(pgd, base, CW)
                for f in range(CW // P):
                    kidx = 2 * c + f
                    pending.append(
                        make_transpose(inter[:, f * P:(f + 1) * P], kidx))
                    pending.append(make_up(c, kidx, U_OFF + f * HIDDEN, 0))
                    pending.append(make_up(c, kidx, U_OFF + f * HIDDEN, 1))

            # chunk 10: two independent 128-wide halves so the chain behind
            # the last weight byte is as short as possible.
            wt_cur = wt[10]
            inter_h = []
            for h in range(2):
                base = 32 if h == 0 else 64
                todo, pending = pending, []
                # h1 pops its 3 deferred ops late (hc>=5): h0's mul is only
                # ~0.9 us behind, and popping early would stall the chain.
                pgd = gd_chain(
                    lambda hc, h=h: (h * 2048 + hc * CW, CW), base, todo,
                    m=1, ltlo=3 * 512 + h * CW,
                    first_pop=(0 if h == 0 else 5))
                inter_h.append(silu_mul(pgd, base, P))
                kidx = 2 * NFULL + h
                if h == 0:
                    pending.append(make_transpose(inter_h[0][:], kidx))
                    pending.append(make_up(10, kidx, U_OFF, 0))
                    pending.append(make_up(10, kidx, U_OFF, 1))
            # final strip: emit inline, jb0 then jb1, so the jb0 copy (DVE)
            # overlaps the jb1 matmul.
            kidx = 2 * NFULL + 1
            make_transpose(inter_h[1][:], kidx)()
            make_up(10, kidx, U_OFF + HIDDEN, 0)()
            make_up(10, kidx, U_OFF + HIDDEN, 1)()
            assert not pending
            assert up_count == [N_UP, N_UP], up_count

            # Final PSUM->SBUF copies run in parallel on DVE (jb0) and the
            # Scalar engine (jb1); each half then goes out via its own
            # HWDGE DMA (Sync and Scalar respectively) so the two output
            # transfers and their ~1.5 us completion latencies overlap.
            out_sb = op.tile([T, HIDDEN], f32)
            nc.vector.tensor_copy(out_sb[:, 0:512], psum_out[:, 0:512])
            nc.sync.dma_start(out_d.ap()[:, 0:512], out_sb[:, 0:512])
            nc.scalar.activation(out_sb[:, 512:1024], psum_out[:, 512:1024],
                                 mybir.ActivationFunctionType.Copy)
            nc.scalar.dma_start(out_d.ap()[:, 512:1024],
                                out_sb[:, 512:1024])

    nc.compile()
    return nc


def _get_compiled():
    global _COMPILED
    if _COMPILED is None:
        _COMPILED = _build()
    return _COMPILED


def _pack_inputs(x, gate_proj, up_proj, down_proj):
    HM = HC * PG  # 992 hidden rows on the main 124-partition strips
    x = np.ascontiguousarray(x, dtype=np.float32)
    # xt[p, s*T + t] = x[t, s*124 + p]
    xt = np.ascontiguousarray(
        x[:, 0:HM].T.reshape(HC, PG, T).transpose(1, 0, 2).reshape(PG, HC * T)
    ).astype(np.float16)
    # xtl[32m + j, t] = x[t, 992 + j], replicated for m=0,1,2 so the
    # leftover matmul's stationary matches any 32m partition offset.
    xtl = np.ascontiguousarray(
        np.tile(x[:, HM:HIDDEN].T, (3, 1))).astype(np.float16)
    eye = np.eye(T, dtype=np.float16)
    in_maps = []
    for k in range(N_CORES):
        g = np.asarray(gate_proj[k], dtype=np.float32)
        d = np.asarray(down_proj[k], dtype=np.float32)
        u = np.asarray(up_proj[k], dtype=np.float32)
        w = np.zeros((NCHUNK, P, WCOLS), dtype=np.float16)
        # Full chunks 0..9 gd: w[c, p, s*512 + o] = g/d[c*CW + o', s*124+p]
        # (strips s of 124 hidden rows; [g 256 | d 256] interleave per s).
        gm = g[:, 0:HM].reshape(NCHUNK, CW, HC, PG).transpose(0, 3, 2, 1)
        dm = d[:, 0:HM].reshape(NCHUNK, CW, HC, PG).transpose(0, 3, 2, 1)
        gdm = np.concatenate([gm, dm], axis=3).reshape(NCHUNK, PG, U_OFF)
        w[:, 0:PG, 0:U_OFF] = gdm
        # Chunk 10 split gd layout: w[10, p, h*2048 + s*256 + o]:
        #   o<128: g[10*CW + h*128 + o, s*124+p]; o>=128: d[.. o-128 ..]
        c = NCHUNK - 1
        gl = g[c * CW:, 0:HM].reshape(2, P, HC, PG).transpose(0, 2, 3, 1)
        dl = d[c * CW:, 0:HM].reshape(2, P, HC, PG).transpose(0, 2, 3, 1)
        gdl = np.concatenate([gl, dl], axis=3)  # [2, HC, PG, 256]
        w[c, 0:PG, 0:U_OFF] = gdl.transpose(2, 0, 1, 3).reshape(PG, U_OFF)
        # Up blocks (all chunks): w[c, p, U_OFF + f*HIDDEN + j] =
        #   u[j, c*CW + f*128 + p]
        w[:, :, U_OFF:] = u.reshape(HIDDEN, NCHUNK, CW // P, P).transpose(
            1, 3, 2, 0).reshape(NCHUNK, P, 2 * HIDDEN)
        # Leftover tile: hidden rows 992..1023 for every chunk. Chunk
        # c = 3b + m sits at partitions [32m, 32m+32), cols [512b, 512b+512)
        # with the same [g|d] interleave as its chain ([g256|d256] for full
        # chunks; [g128|d128] per half for chunk 10 at b=3: half h at cols
        # [1536 + h*256, ...)).
        ltile = np.zeros((96, 4 * 512), dtype=np.float16)
        gr = g[:, HM:HIDDEN]  # [INTER, 32]
        dr = d[:, HM:HIDDEN]
        for c2 in range(NFULL):
            b, m = divmod(c2, 3)
            blk = np.concatenate(
                [gr[c2 * CW:(c2 + 1) * CW], dr[c2 * CW:(c2 + 1) * CW]],
                axis=0)  # [512, 32]
            ltile[32 * m:32 * m + 32, 512 * b:512 * b + 512] = blk.T
        for h in range(2):
            lo = c * CW + h * P
            blk = np.concatenate([gr[lo:lo + P], dr[lo:lo + P]], axis=0)
            ltile[32:64, 3 * 512 + h * CW:3 * 512 + (h + 1) * CW] = blk.T
        in_maps.append({"xt": xt, "xtl": xtl, "eye": eye, "lt": ltile,
                        "w": np.ascontiguousarray(w)})
    return in_maps


def kernel(x, expert_indices, gate_proj, up_proj, down_proj):
    global LAST_RESULTS
    from concourse.bass_utils import run_bass_kernel_spmd

    nc = _get_compiled()
    in_maps = _pack_inputs(x, gate_proj, up_proj, down_proj)
    res = run_bass_kernel_spmd(nc, in_maps, core_ids=list(range(N_CORES)),
                               trace=TRACE)
    LAST_RESULTS = res

    expert_outs = np.stack([res.results[k]["out"] for k in range(N_CORES)])
    idx = np.asarray(expert_indices).astype(np.int64)  # [T, TOP_K]
    return expert_outs[idx, np.arange(T)[:, None], :].astype(np.float32)


# revision 15
# speedup vs baseline: 1.6638x; 1.6638x over previous
"""Expert-parallel MoE conditional feed-forward for 8 Trainium2 NeuronCores.

Problem: x[16,1024], expert_indices[16,2], gate/down_proj[8,2816,1024],
up_proj[8,1024,2816]. Reference computes, per (token, slot) pair with
e = expert_indices[t, a]:
    out[t,a,:] = (silu(x @ gate_proj[e].T) * (x @ down_proj[e].T)) @ up_proj[e].T

Sharding: core k owns expert k and computes its FFN output for ALL 16
tokens (the compute is negligible; the kernel is weight-streaming bound).
The host then gathers rows per expert_indices. This needs no indices on
device and is load-balanced regardless of routing.

Weights and x are cast to fp16 on the host (harness gate is 2e-2; fp16
end-to-end measures 4.7e-4 while fp8 e4m3 is mantissa-limited at 2.7e-2+
per matrix). 17.3 MB per core streams at the ~420 GB/s per-core fabric
ceiling measured when all 8 cores stream (~42 us).

Timeline anatomy (measured): exec_time = [first GpSimd MEMSET ... last
epilogue NOTIFY]. A ~5.8 us start rendezvous is excluded; a fixed
~8.4 us walrus epilogue (per-semaphore zeroing avalanche) is included.
So the controllable part is first-DMA-issue -> out-DMA-complete.

Key structural choices vs the previous revision:
  * Weight chunk 0's first DMA is issued BEFORE xt/eye so streaming (the
    critical path) starts ~1.3 us earlier.
  * Chunks are processed singly (not in pairs): chunk c's 4 up-matmuls
    are deferred only into chunk c+1's gate/down chain. That keeps the
    end-of-stream backlog to one chunk.
  * Chunks 0, 9, 10 stream as split DMAs; Tile's range-level dependency
    tracking lets consuming matmuls start as soon as their slice lands.
    Chunk 10 additionally uses a per-half column layout and is processed
    as two independent 128-wide half-chunks, so the dependency chain
    behind the very last weight byte is one 8-matmul N=256 chain + one
    silu/mul/transpose + 2 up-matmuls.
  * The final PSUM->SBUF copies run in parallel (jb0 on DVE, jb1 on the
    Scalar engine as an activation-Copy) before a single output DMA.

PE scheduling (array packing via PSUM base partition; q3 unusable per
HW bug): q1 (psum rows 32-47) and q2 (rows 64-79) alternate per chunk
for the gate|down chains; q0 (rows 0-15) carries all up-projection
accumulation into psum_out. The [16,128] fp16 intermediates are
transposed to [128,16] via identity matmuls on the PE, cast to fp16 on
the PSUM->SBUF copy, and fed as stationaries.
"""

import sys

for _p in ("/opt/trn_rl_repo", "/opt/pypackages"):
    if _p not in sys.path:
        sys.path.append(_p)

import numpy as np

NUM_EXPERTS = 8
HIDDEN = 1024
INTER = 2816
T = 16
N_CORES = 8
P = 128
PG = 124                  # gate/down weights use partitions 0..123 only:
                          # SDMA engine 15 (the chronically contended one)
                          # serves partitions {92-95, 124-127}; dropping
                          # 124-127 halves its load so it never limits the
                          # stream. Hidden rows 992..1023 go to a shared
                          # "leftover" tile on partitions 0..95.
CW = 256                  # intermediate chunk width
NCHUNK = INTER // CW      # 11
NFULL = NCHUNK - 1        # chunks 0..9 use the full-chunk layout
HC = HIDDEN // P          # 8 strips per gate/down chain (of PG rows each)
U_OFF = 2 * HC * CW       # 4096: offset of up blocks in packed W
WCOLS = U_OFF + 2 * HIDDEN  # 6144
N_UP = 2 * NFULL + 2      # 22 up-matmuls per 512-col output half

_COMPILED = None
LAST_RESULTS = None
TRACE = False


def _build():
    import concourse.bacc as bacc
    import concourse.bass as bass
    import concourse.tile as tile
    from concourse import mybir

    f32 = mybir.dt.float32
    f16 = mybir.dt.float16
    nc = bacc.Bacc("TRN2", target_bir_lowering=False, debug=False,
                   num_devices=N_CORES)
    xt_d = nc.dram_tensor("xt", [PG, HC * T], f16, kind="ExternalInput")
    xtl_d = nc.dram_tensor("xtl", [96, T], f16, kind="ExternalInput")
    eye_d = nc.dram_tensor("eye", [T, T], f16, kind="ExternalInput")
    lt_d = nc.dram_tensor("lt", [96, 4 * 512], f16, kind="ExternalInput")
    w_d = nc.dram_tensor("w", [NCHUNK, P, WCOLS], f16, kind="ExternalInput")
    out_d = nc.dram_tensor("out", [T, HIDDEN], f32, kind="ExternalOutput")

    with tile.TileContext(nc) as tc:
        with (
            tc.tile_pool(name="xp", bufs=1) as xp,
            tc.tile_pool(name="wp", bufs=1) as wp,
            tc.tile_pool(name="ip", bufs=4) as ip,
            tc.tile_pool(name="itp", bufs=1) as itp,
            tc.tile_pool(name="pg", bufs=3, space=bass.MemorySpace.PSUM) as pgp,
            tc.tile_pool(name="tp", bufs=2, space=bass.MemorySpace.PSUM) as tpp,
            tc.tile_pool(name="po", bufs=1, space=bass.MemorySpace.PSUM) as pop,
            tc.tile_pool(name="op", bufs=1) as op,
        ):
            xt = xp.tile([PG, HC * T], f16)
            xtl = xp.tile([96, T], f16)
            eye = xp.tile([T, T], f16)
            lt = xp.tile([96, 4 * 512], f16)
            wt = [wp.tile([P, WCOLS], f16, name=f"w{c}", tag=f"w{c}")
                  for c in range(NCHUNK)]

            # xt/xtl/eye go via GpSimd (SWDGE) so the Sync engine issues
            # only weight DMAs back-to-back (descriptor generation is
            # ~650 ns per dma_start; keeping Sync weight-only shortens the
            # stream ramp). Gate/down DMAs move partitions 0..123 only;
            # the leftover tile lt carries hidden rows 992..1023 for all
            # chunks and lands early. Chunks 0, 9, 10 are split so
            # consuming matmuls start per-slice (Tile tracks range-level
            # DMA->reader deps).
            nc.gpsimd.dma_start(xt[:], xt_d.ap())
            nc.gpsimd.dma_start(xtl[:], xtl_d.ap())
            nc.gpsimd.dma_start(eye[:], eye_d.ap())

            # DMA descriptors are dealt to SDMA engines as equal contiguous
            # partition blocks using the largest divisor of the partition
            # count <= 16 (measured: a [124, X] DMA collapses to 4 engines
            # of 31 rows!). So every gd DMA is issued as [0:64] (16 engines
            # x 4 rows) + [64:124] (60 rows -> 15 engines x 4 rows, engine
            # 15 excluded). Net per-engine load: engines 0-14 exactly as in
            # the balanced 128-row layout, engine 15 (the chronically
            # contended one) at half load so it never limits the stream.
            def gd_dma(c, lo, hi):
                nc.sync.dma_start(wt[c][0:64, lo:hi],
                                  w_d.ap()[c][0:64, lo:hi])
                nc.sync.dma_start(wt[c][64:PG, lo:hi],
                                  w_d.ap()[c][64:PG, lo:hi])

            gd_dma(0, 0, U_OFF // 2)
            nc.sync.dma_start(lt[:], lt_d.ap())
            gd_dma(0, U_OFF // 2, U_OFF)
            nc.sync.dma_start(wt[0][:, U_OFF:WCOLS],
                              w_d.ap()[0][:, U_OFF:WCOLS])
            for c in range(1, NFULL):
                gd_dma(c, 0, U_OFF)
                nc.sync.dma_start(wt[c][:, U_OFF:WCOLS],
                                  w_d.ap()[c][:, U_OFF:WCOLS])
            # chunk 10 (split layout): half h gd at [h*2048,(h+1)*2048),
            # half h up at [4096+h*1024, 4096+(h+1)*1024)
            gd_dma(10, 0, 2048)
            nc.sync.dma_start(wt[10][:, U_OFF:U_OFF + HIDDEN],
                              w_d.ap()[10][:, U_OFF:U_OFF + HIDDEN])
            gd_dma(10, 2048, U_OFF)
            nc.sync.dma_start(wt[10][:, U_OFF + HIDDEN:WCOLS],
                              w_d.ap()[10][:, U_OFF + HIDDEN:WCOLS])

            psum_out = pop.tile([T, HIDDEN], f32)
            itall = itp.tile([P, N_UP * T], f16)
            up_count = [0, 0]    # per-jb position in the accumulation chain
            pending = []         # PE thunks deferred from the previous chunk

            # All PE work downstream of a chunk's silu/mul (the transpose
            # and the two-or-four up-matmuls) is deferred into the NEXT
            # chunk's gate/down chain. The PE is strictly in-order, so
            # emitting a transpose right after its own chunk's chain would
            # stall the PE queue ~1.2 us waiting on ACT/DVE; one chunk
            # later the operands are long ready.
            def make_transpose(inter_slice, kidx):
                def t():
                    tp = tpp.tile([P, T], f32, name="tp")
                    nc.tensor.matmul(tp[:], inter_slice, eye[:])
                    nc.vector.tensor_copy(itall[:, kidx * T:(kidx + 1) * T],
                                          tp[:])
                return t

            def make_up(c, kidx, upbase, jb):
                def u():
                    k = up_count[jb]
                    up_count[jb] += 1
                    nc.tensor.matmul(
                        psum_out[:, jb * 512:(jb + 1) * 512],
                        itall[:, kidx * T:(kidx + 1) * T],
                        wt[c][:, upbase + jb * 512:upbase + (jb + 1) * 512],
                        start=(k == 0), stop=(k == N_UP - 1),
                    )
                return u

            def gd_chain(cols_of_hc, base, todo, m, ltlo, first_pop=0):
                # 8 strips of PG=124 hidden rows from the chunk's own tile,
                # then the 32-row leftover strip (hidden 992..1023) from
                # the shared lt tile at partition offset 32*m.
                pgd = pgp.tile([P, 2 * CW], f32, name="pgd")
                for hc in range(HC):
                    lo, width = cols_of_hc(hc)
                    nc.tensor.matmul(
                        pgd[base:base + T, 0:width],
                        xt[:, hc * T:(hc + 1) * T],
                        wt_cur[0:PG, lo:lo + width],
                        start=(hc == 0), stop=False,
                    )
                    if todo and hc >= first_pop:
                        todo.pop(0)()
                lo, width = cols_of_hc(0)
                nc.tensor.matmul(
                    pgd[base:base + T, 0:width],
                    xtl[32 * m:32 * m + 32, :],
                    lt[32 * m:32 * m + 32, ltlo:ltlo + width],
                    start=False, stop=True,
                )
                while todo:
                    todo.pop(0)()
                return pgd

            def silu_mul(pgd, base, width):
                s1 = ip.tile([T, width], f32, name="s1")
                nc.scalar.activation(s1[:], pgd[base:base + T, 0:width],
                                     mybir.ActivationFunctionType.Silu)
                inter = ip.tile([T, width], f16, name="inter")
                nc.vector.tensor_mul(inter[:], s1[:],
                                     pgd[base:base + T, width:2 * width])
                return inter

            for c in range(NFULL):
                base = 32 if c % 2 == 0 else 64
                wt_cur = wt[c]
                todo, pending = pending, []
                pgd = gd_chain(lambda hc: (hc * 2 * CW, 2 * CW), base, todo,
                               m=c % 3, ltlo=512 * (c // 3))
                inter = silu_mul(pgd, base, CW)
                for f in range(CW // P):
                    kidx = 2 * c + f
                    pending.append(
                        make_transpose(inter[:, f * P:(f + 1) * P], kidx))
                    pending.append(make_up(c, kidx, U_OFF + f * HIDDEN, 0))
                    pending.append(make_up(c, kidx, U_OFF + f * HIDDEN, 1))

            # chunk 10: two independent 128-wide halves so the chain behind
            # the last weight byte is as short as possible.
            wt_cur = wt[10]
            inter_h = []
            for h in range(2):
                base = 32 if h == 0 else 64
                todo, pending = pending, []
                # h1 pops its 3 deferred ops late (hc>=5): h0's mul is only
                # ~0.9 us behind, and popping early would stall the chain.
                pgd = gd_chain(
                    lambda hc, h=h: (h * 2048 + hc * CW, CW), base, todo,
                    m=1, ltlo=3 * 512 + h * CW,
                    first_pop=(0 if h == 0 else 5))
                inter_h.append(silu_mul(pgd, base, P))
                kidx = 2 * NFULL + h
                if h == 0:
                    pending.append(make_transpose(inter_h[0][:], kidx))
                    pending.append(make_up(10, kidx, U_OFF, 0))
                    pending.append(make_up(10, kidx, U_OFF, 1))
            # final strip: emit inline, jb0 then jb1, so the jb0 copy (DVE)
            # overlaps the jb1 matmul.
            kidx = 2 * NFULL + 1
            make_transpose(inter_h[1][:], kidx)()
            make_up(10, kidx, U_OFF + HIDDEN, 0)()
            make_up(10, kidx, U_OFF + HIDDEN, 1)()
            assert not pending
            assert up_count == [N_UP, N_UP], up_count

            # Final PSUM->SBUF copies run in parallel on DVE (jb0) and the
            # Scalar engine (jb1); each half then goes out via its own
            # HWDGE DMA (Sync and Scalar respectively) so the two output
            # transfers and their ~1.5 us completion latencies overlap.
            out_sb = op.tile([T, HIDDEN], f32)
            nc.vector.tensor_copy(out_sb[:, 0:512], psum_out[:, 0:512])
            nc.sync.dma_start(out_d.ap()[:, 0:512], out_sb[:, 0:512])
            nc.scalar.activation(out_sb[:, 512:1024], psum_out[:, 512:1024],
                                 mybir.ActivationFunctionType.Copy)
            nc.scalar.dma_start(out_d.ap()[:, 512:1024],
                                out_sb[:, 512:1024])

    nc.compile()
    return nc


def _get_compiled():
    global _COMPILED
    if _COMPILED is None:
        _COMPILED = _build()
    return _COMPILED


def _pack_inputs(x, gate_proj, up_proj, down_proj):
    HM = HC * PG  # 992 hidden rows on the main 124-partition strips
    x = np.ascontiguousarray(x, dtype=np.float32)
    # xt[p, s*T + t] = x[t, s*124 + p]
    xt = np.ascontiguousarray(
        x[:, 0:HM].T.reshape(HC, PG, T).transpose(1, 0, 2).reshape(PG, HC * T)
    ).astype(np.float16)
    # xtl[32m + j, t] = x[t, 992 + j], replicated for m=0,1,2 so the
    # leftover matmul's stationary matches any 32m partition offset.
    xtl = np.ascontiguousarray(
        np.tile(x[:, HM:HIDDEN].T, (3, 1))).astype(np.float16)
    eye = np.eye(T, dtype=np.float16)
    in_maps = []
    for k in range(N_CORES):
        g = np.asarray(gate_proj[k], dtype=np.float32)
        d = np.asarray(down_proj[k], dtype=np.float32)
        u = np.asarray(up_proj[k], dtype=np.float32)
        w = np.zeros((NCHUNK, P, WCOLS), dtype=np.float16)
        # Full chunks 0..9 gd: w[c, p, s*512 + o] = g/d[c*CW + o', s*124+p]
        # (strips s of 124 hidden rows; [g 256 | d 256] interleave per s).
        gm = g[:, 0:HM].reshape(NCHUNK, CW, HC, PG).transpose(0, 3, 2, 1)
        dm = d[:, 0:HM].reshape(NCHUNK, CW, HC, PG).transpose(0, 3, 2, 1)
        gdm = np.concatenate([gm, dm], axis=3).reshape(NCHUNK, PG, U_OFF)
        w[:, 0:PG, 0:U_OFF] = gdm
        # Chunk 10 split gd layout: w[10, p, h*2048 + s*256 + o]:
        #   o<128: g[10*CW + h*128 + o, s*124+p]; o>=128: d[.. o-128 ..]
        c = NCHUNK - 1
        gl = g[c * CW:, 0:HM].reshape(2, P, HC, PG).transpose(0, 2, 3, 1)
        dl = d[c * CW:, 0:HM].reshape(2, P, HC, PG).transpose(0, 2, 3, 1)
        gdl = np.concatenate([gl, dl], axis=3)  # [2, HC, PG, 256]
        w[c, 0:PG, 0:U_OFF] = gdl.transpose(2, 0, 1, 3).reshape(PG, U_OFF)
        # Up blocks (all chunks): w[c, p, U_OFF + f*HIDDEN + j] =
        #   u[j, c*CW + f*128 + p]
        w[:, :, U_OFF:] = u.reshape(HIDDEN, NCHUNK, CW // P, P).transpose(
            1, 3, 2, 0).reshape(NCHUNK, P, 2 * HIDDEN)
        # Leftover tile: hidden rows 992..1023 for every chunk. Chunk
        # c = 3b + m sits at partitions [32m, 32m+32), cols [512b, 512b+512)
        # with the same [g|d] interleave as its chain ([g256|d256] for full
        # chunks; [g128|d128] per half for chunk 10 at b=3: half h at cols
        # [1536 + h*256, ...)).
        ltile = np.zeros((96, 4 * 512), dtype=np.float16)
        gr = g[:, HM:HIDDEN]  # [INTER, 32]
        dr = d[:, HM:HIDDEN]
        for c2 in range(NFULL):
            b, m = divmod(c2, 3)
            blk = np.concatenate(
                [gr[c2 * CW:(c2 + 1) * CW], dr[c2 * CW:(c2 + 1) * CW]],
                axis=0)  # [512, 32]
            ltile[32 * m:32 * m + 32, 512 * b:512 * b + 512] = blk.T
        for h in range(2):
            lo = c * CW + h * P
            blk = np.concatenate([gr[lo:lo + P], dr[lo:lo + P]], axis=0)
            ltile[32:64, 3 * 512 + h * CW:3 * 512 + (h + 1) * CW] = blk.T
        in_maps.append({"xt": xt, "xtl": xtl, "eye": eye, "lt": ltile,
                        "w": np.ascontiguousarray(w)})
    return in_maps


def kernel(x, expert_indices, gate_proj, up_proj, down_proj):
    global LAST_RESULTS
    from concourse.bass_utils import run_bass_kernel_spmd

    nc = _get_compiled()
    in_maps = _pack_inputs(x, gate_proj, up_proj, down_proj)
    res = run_bass_kernel_spmd(nc, in_maps, core_ids=list(range(N_CORES)),
                               trace=TRACE)
    LAST_RESULTS = res

    expert_outs = np.stack([res.results[k]["out"] for k in range(N_CORES)])
    idx = np.asarray(expert_indices).astype(np.int64)  # [T, TOP_K]
    return expert_outs[idx, np.arange(T)[:, None], :].astype(np.float32)


# revision 18
# speedup vs baseline: 1.7143x; 1.0304x over previous
"""Expert-parallel MoE conditional feed-forward for 8 Trainium2 NeuronCores.

Problem: x[16,1024], expert_indices[16,2], gate/down_proj[8,2816,1024],
up_proj[8,1024,2816]. Reference computes, per (token, slot) pair with
e = expert_indices[t, a]:
    out[t,a,:] = (silu(x @ gate_proj[e].T) * (x @ down_proj[e].T)) @ up_proj[e].T

Sharding: core k owns expert k and computes its FFN output for ALL 16
tokens (the compute is negligible; the kernel is weight-streaming bound).
The host then gathers rows per expert_indices. This needs no indices on
device and is load-balanced regardless of routing.

Weights and x are cast to fp16 on the host (harness gate is 2e-2; fp16
end-to-end measures 4.7e-4 while fp8 e4m3 is mantissa-limited at 2.7e-2+
per matrix). 17.3 MB per core streams at the ~420 GB/s per-core fabric
ceiling measured when all 8 cores stream (~42 us).

Timeline anatomy (measured): exec_time = [first GpSimd MEMSET ... last
epilogue NOTIFY]. A ~5.8 us start rendezvous is excluded; a fixed
~8.4 us walrus epilogue (per-semaphore zeroing avalanche) is included.
So the controllable part is first-DMA-issue -> out-DMA-complete.

Key structural choices vs the previous revision:
  * Weight chunk 0's first DMA is issued BEFORE xt/eye so streaming (the
    critical path) starts ~1.3 us earlier.
  * Chunks are processed singly (not in pairs): chunk c's 4 up-matmuls
    are deferred only into chunk c+1's gate/down chain. That keeps the
    end-of-stream backlog to one chunk.
  * Chunks 0, 9, 10 stream as split DMAs; Tile's range-level dependency
    tracking lets consuming matmuls start as soon as their slice lands.
    Chunk 10 additionally uses a per-half column layout and is processed
    as two independent 128-wide half-chunks, so the dependency chain
    behind the very last weight byte is one 8-matmul N=256 chain + one
    silu/mul/transpose + 2 up-matmuls.
  * The final PSUM->SBUF copies run in parallel (jb0 on DVE, jb1 on the
    Scalar engine as an activation-Copy) before a single output DMA.

PE scheduling (array packing via PSUM base partition; q3 unusable per
HW bug): q1 (psum rows 32-47) and q2 (rows 64-79) alternate per chunk
for the gate|down chains; q0 (rows 0-15) carries all up-projection
accumulation into psum_out. The [16,128] fp16 intermediates are
transposed to [128,16] via identity matmuls on the PE, cast to fp16 on
the PSUM->SBUF copy, and fed as stationaries.
"""

import sys

for _p in ("/opt/trn_rl_repo", "/opt/pypackages"):
    if _p not in sys.path:
        sys.path.append(_p)

import numpy as np

NUM_EXPERTS = 8
HIDDEN = 1024
INTER = 2816
T = 16
N_CORES = 8
P = 128
PG = 120                  # gate/down weights use partitions 0..119 only.
                          # DMA descriptors are dealt to SDMA engines as
                          # equal contiguous partition blocks (largest
                          # divisor of the partition count <= 16), and an
                          # engine only hits full rate when its block spans
                          # >= 8 partitions (2 AXI ports). [120, X] DMAs ->
                          # 15 engines x 8 rows at full rate, with engine
                          # 15 (the chronically contended one) carrying no
                          # gate/down bytes at all. Hidden rows 960..1023
                          # go to a shared "leftover" tile.
CW = 256                  # intermediate chunk width
NCHUNK = INTER // CW      # 11
NFULL = NCHUNK - 1        # chunks 0..9 use the full-chunk layout
HC = HIDDEN // P          # 8 strips per gate/down chain (of PG rows each)
U_OFF = 2 * HC * CW       # 4096: offset of up blocks in packed W
WCOLS = U_OFF + 2 * HIDDEN  # 6144
N_UP = 2 * NFULL + 2      # 22 up-matmuls per 512-col output half

_COMPILED = None
LAST_RESULTS = None
TRACE = False


def _build():
    import concourse.bacc as bacc
    import concourse.bass as bass
    import concourse.tile as tile
    from concourse import mybir

    f32 = mybir.dt.float32
    f16 = mybir.dt.float16
    nc = bacc.Bacc("TRN2", target_bir_lowering=False, debug=False,
                   num_devices=N_CORES)
    xt_d = nc.dram_tensor("xt", [PG, HC * T], f16, kind="ExternalInput")
    xtl_d = nc.dram_tensor("xtl", [P, T], f16, kind="ExternalInput")
    eye_d = nc.dram_tensor("eye", [T, T], f16, kind="ExternalInput")
    lt_d = nc.dram_tensor("lt", [P, 6 * 512], f16, kind="ExternalInput")
    w_d = nc.dram_tensor("w", [NCHUNK, P, WCOLS], f16, kind="ExternalInput")
    out_d = nc.dram_tensor("out", [T, HIDDEN], f32, kind="ExternalOutput")

    with tile.TileContext(nc) as tc:
        with (
            tc.tile_pool(name="xp", bufs=1) as xp,
            tc.tile_pool(name="wp", bufs=1) as wp,
            tc.tile_pool(name="ip", bufs=4) as ip,
            tc.tile_pool(name="itp", bufs=1) as itp,
            tc.tile_pool(name="pg", bufs=3, space=bass.MemorySpace.PSUM) as pgp,
            tc.tile_pool(name="tp", bufs=2, space=bass.MemorySpace.PSUM) as tpp,
            tc.tile_pool(name="po", bufs=1, space=bass.MemorySpace.PSUM) as pop,
            tc.tile_pool(name="op", bufs=1) as op,
        ):
            xt = xp.tile([PG, HC * T], f16)
            xtl = xp.tile([P, T], f16)
            eye = xp.tile([T, T], f16)
            lt = xp.tile([P, 6 * 512], f16)
            wt = [wp.tile([P, WCOLS], f16, name=f"w{c}", tag=f"w{c}")
                  for c in range(NCHUNK)]

            # xt/xtl/eye go via GpSimd (SWDGE) so the Sync engine issues
            # only weight DMAs back-to-back (descriptor generation is
            # ~650 ns per dma_start; keeping Sync weight-only shortens the
            # stream ramp). Gate/down DMAs move partitions 0..123 only;
            # the leftover tile lt carries hidden rows 992..1023 for all
            # chunks and lands early. Chunks 0, 9, 10 are split so
            # consuming matmuls start per-slice (Tile tracks range-level
            # DMA->reader deps).
            nc.gpsimd.dma_start(xt[:], xt_d.ap())
            nc.gpsimd.dma_start(xtl[:], xtl_d.ap())
            nc.gpsimd.dma_start(eye[:], eye_d.ap())

            def gd_dma(c, lo, hi):
                nc.sync.dma_start(wt[c][0:PG, lo:hi],
                                  w_d.ap()[c][0:PG, lo:hi])

            gd_dma(0, 0, U_OFF // 2)
            nc.sync.dma_start(lt[:], lt_d.ap())
            gd_dma(0, U_OFF // 2, U_OFF)
            nc.sync.dma_start(wt[0][:, U_OFF:WCOLS],
                              w_d.ap()[0][:, U_OFF:WCOLS])
            for c in range(1, NFULL):
                gd_dma(c, 0, U_OFF)
                nc.sync.dma_start(wt[c][:, U_OFF:WCOLS],
                                  w_d.ap()[c][:, U_OFF:WCOLS])
            # chunk 10 (split layout): half h gd at [h*2048,(h+1)*2048),
            # half h up at [4096+h*1024, 4096+(h+1)*1024)
            gd_dma(10, 0, 2048)
            nc.sync.dma_start(wt[10][:, U_OFF:U_OFF + HIDDEN],
                              w_d.ap()[10][:, U_OFF:U_OFF + HIDDEN])
            gd_dma(10, 2048, U_OFF)
            nc.sync.dma_start(wt[10][:, U_OFF + HIDDEN:WCOLS],
                              w_d.ap()[10][:, U_OFF + HIDDEN:WCOLS])

            psum_out = pop.tile([T, HIDDEN], f32)
            itall = itp.tile([P, N_UP * T], f16)
            up_count = [0, 0]    # per-jb position in the accumulation chain
            pending = []         # PE thunks deferred from the previous chunk

            # All PE work downstream of a chunk's silu/mul (the transpose
            # and the two-or-four up-matmuls) is deferred into the NEXT
            # chunk's gate/down chain. The PE is strictly in-order, so
            # emitting a transpose right after its own chunk's chain would
            # stall the PE queue ~1.2 us waiting on ACT/DVE; one chunk
            # later the operands are long ready.
            def make_transpose(inter_slice, kidx):
                def t():
                    tp = tpp.tile([P, T], f32, name="tp")
                    nc.tensor.matmul(tp[:], inter_slice, eye[:])
                    nc.vector.tensor_copy(itall[:, kidx * T:(kidx + 1) * T],
                                          tp[:])
                return t

            def make_up(c, kidx, upbase, jb):
                def u():
                    k = up_count[jb]
                    up_count[jb] += 1
                    nc.tensor.matmul(
                        psum_out[:, jb * 512:(jb + 1) * 512],
                        itall[:, kidx * T:(kidx + 1) * T],
                        wt[c][:, upbase + jb * 512:upbase + (jb + 1) * 512],
                        start=(k == 0), stop=(k == N_UP - 1),
                    )
                return u

            def gd_chain(cols_of_hc, base, todo, m, ltlo, first_pop=0):
                # 8 strips of PG=120 hidden rows from the chunk's own tile,
                # then the 64-row leftover strip (hidden 960..1023) from
                # the shared lt tile at partition offset 64*m.
                pgd = pgp.tile([P, 2 * CW], f32, name="pgd")
                for hc in range(HC):
                    lo, width = cols_of_hc(hc)
                    nc.tensor.matmul(
                        pgd[base:base + T, 0:width],
                        xt[:, hc * T:(hc + 1) * T],
                        wt_cur[0:PG, lo:lo + width],
                        start=(hc == 0), stop=False,
                    )
                    if todo and hc >= first_pop:
                        todo.pop(0)()
                lo, width = cols_of_hc(0)
                nc.tensor.matmul(
                    pgd[base:base + T, 0:width],
                    xtl[64 * m:64 * m + 64, :],
                    lt[64 * m:64 * m + 64, ltlo:ltlo + width],
                    start=False, stop=True,
                )
                while todo:
                    todo.pop(0)()
                return pgd

            def silu_mul(pgd, base, width):
                s1 = ip.tile([T, width], f32, name="s1")
                nc.scalar.activation(s1[:], pgd[base:base + T, 0:width],
                                     mybir.ActivationFunctionType.Silu)
                inter = ip.tile([T, width], f16, name="inter")
                nc.vector.tensor_mul(inter[:], s1[:],
                                     pgd[base:base + T, width:2 * width])
                return inter

            for c in range(NFULL):
                base = 32 if c % 2 == 0 else 64
                wt_cur = wt[c]
                todo, pending = pending, []
                pgd = gd_chain(lambda hc: (hc * 2 * CW, 2 * CW), base, todo,
                               m=c % 2, ltlo=512 * (c // 2))
                inter = silu_mul(pgd, base, CW)
                for f in range(CW // P):
                    kidx = 2 * c + f
                    pending.append(
                        make_transpose(inter[:, f * P:(f + 1) * P], kidx))
                    pending.append(make_up(c, kidx, U_OFF + f * HIDDEN, 0))
                    pending.append(make_up(c, kidx, U_OFF + f * HIDDEN, 1))

            # chunk 10: two independent 128-wide halves so the chain behind
            # the last weight byte is as short as possible.
            wt_cur = wt[10]
            inter_h = []
            for h in range(2):
                base = 32 if h == 0 else 64
                todo, pending = pending, []
                # h1 pops its 3 deferred ops late (hc>=5): h0's mul is only
                # ~0.9 us behind, and popping early would stall the chain.
                pgd = gd_chain(
                    lambda hc, h=h: (h * 2048 + hc * CW, CW), base, todo,
                    m=0, ltlo=5 * 512 + h * CW,
                    first_pop=(0 if h == 0 else 5))
                inter_h.append(silu_mul(pgd, base, P))
                kidx = 2 * NFULL + h
                if h == 0:
                    pending.append(make_transpose(inter_h[0][:], kidx))
                    pending.append(make_up(10, kidx, U_OFF, 0))
                    pending.append(make_up(10, kidx, U_OFF, 1))
            # final strip: emit inline, jb0 then jb1, so the jb0 copy (DVE)
            # overlaps the jb1 matmul.
            kidx = 2 * NFULL + 1
            make_transpose(inter_h[1][:], kidx)()
            make_up(10, kidx, U_OFF + HIDDEN, 0)()
            make_up(10, kidx, U_OFF + HIDDEN, 1)()
            assert not pending
            assert up_count == [N_UP, N_UP], up_count

            # Final PSUM->SBUF copies run in parallel on DVE (jb0) and the
            # Scalar engine (jb1); each half then goes out via its own
            # HWDGE DMA (Sync and Scalar respectively) so the two output
            # transfers and their ~1.5 us completion latencies overlap.
            out_sb = op.tile([T, HIDDEN], f32)
            nc.vector.tensor_copy(out_sb[:, 0:512], psum_out[:, 0:512])
            nc.sync.dma_start(out_d.ap()[:, 0:512], out_sb[:, 0:512])
            nc.scalar.activation(out_sb[:, 512:1024], psum_out[:, 512:1024],
                                 mybir.ActivationFunctionType.Copy)
            nc.scalar.dma_start(out_d.ap()[:, 512:1024],
                                out_sb[:, 512:1024])

    nc.compile()
    return nc


def _get_compiled():
    global _COMPILED
    if _COMPILED is None:
        _COMPILED = _build()
    return _COMPILED


def _pack_inputs(x, gate_proj, up_proj, down_proj):
    HM = HC * PG  # 992 hidden rows on the main 124-partition strips
    x = np.ascontiguousarray(x, dtype=np.float32)
    # xt[p, s*T + t] = x[t, s*124 + p]
    xt = np.ascontiguousarray(
        x[:, 0:HM].T.reshape(HC, PG, T).transpose(1, 0, 2).reshape(PG, HC * T)
    ).astype(np.float16)
    # xtl[64m + j, t] = x[t, 960 + j], replicated for m=0,1 so the
    # leftover matmul's stationary matches either 64m partition offset.
    xtl = np.ascontiguousarray(
        np.tile(x[:, HM:HIDDEN].T, (2, 1))).astype(np.float16)
    eye = np.eye(T, dtype=np.float16)
    in_maps = []
    for k in range(N_CORES):
        g = np.asarray(gate_proj[k], dtype=np.float32)
        d = np.asarray(down_proj[k], dtype=np.float32)
        u = np.asarray(up_proj[k], dtype=np.float32)
        w = np.zeros((NCHUNK, P, WCOLS), dtype=np.float16)
        # Full chunks 0..9 gd: w[c, p, s*512 + o] = g/d[c*CW + o', s*124+p]
        # (strips s of 124 hidden rows; [g 256 | d 256] interleave per s).
        gm = g[:, 0:HM].reshape(NCHUNK, CW, HC, PG).transpose(0, 3, 2, 1)
        dm = d[:, 0:HM].reshape(NCHUNK, CW, HC, PG).transpose(0, 3, 2, 1)
        gdm = np.concatenate([gm, dm], axis=3).reshape(NCHUNK, PG, U_OFF)
        w[:, 0:PG, 0:U_OFF] = gdm
        # Chunk 10 split gd layout: w[10, p, h*2048 + s*256 + o]:
        #   o<128: g[10*CW + h*128 + o, s*124+p]; o>=128: d[.. o-128 ..]
        c = NCHUNK - 1
        gl = g[c * CW:, 0:HM].reshape(2, P, HC, PG).transpose(0, 2, 3, 1)
        dl = d[c * CW:, 0:HM].reshape(2, P, HC, PG).transpose(0, 2, 3, 1)
        gdl = np.concatenate([gl, dl], axis=3)  # [2, HC, PG, 256]
        w[c, 0:PG, 0:U_OFF] = gdl.transpose(2, 0, 1, 3).reshape(PG, U_OFF)
        # Up blocks (all chunks): w[c, p, U_OFF + f*HIDDEN + j] =
        #   u[j, c*CW + f*128 + p]
        w[:, :, U_OFF:] = u.reshape(HIDDEN, NCHUNK, CW // P, P).transpose(
            1, 3, 2, 0).reshape(NCHUNK, P, 2 * HIDDEN)
        # Leftover tile: hidden rows 960..1023 for every chunk. Chunk
        # c = 2b + m sits at partitions [64m, 64m+64), cols [512b, 512b+512)
        # with the same [g|d] interleave as its chain ([g256|d256] for full
        # chunks; [g128|d128] per half for chunk 10 at b=5, m=0: half h at
        # cols [2560 + h*256, ...)).
        ltile = np.zeros((P, 6 * 512), dtype=np.float16)
        gr = g[:, HM:HIDDEN]  # [INTER, 64]
        dr = d[:, HM:HIDDEN]
        for c2 in range(NFULL):
            b, m = divmod(c2, 2)
            blk = np.concatenate(
                [gr[c2 * CW:(c2 + 1) * CW], dr[c2 * CW:(c2 + 1) * CW]],
                axis=0)  # [512, 64]
            ltile[64 * m:64 * m + 64, 512 * b:512 * b + 512] = blk.T
        for h in range(2):
            lo = c * CW + h * P
            blk = np.concatenate([gr[lo:lo + P], dr[lo:lo + P]], axis=0)
            ltile[0:64, 5 * 512 + h * CW:5 * 512 + (h + 1) * CW] = blk.T
        in_maps.append({"xt": xt, "xtl": xtl, "eye": eye, "lt": ltile,
                        "w": np.ascontiguousarray(w)})
    return in_maps


def kernel(x, expert_indices, gate_proj, up_proj, down_proj):
    global LAST_RESULTS
    from concourse.bass_utils import run_bass_kernel_spmd

    nc = _get_compiled()
    in_maps = _pack_inputs(x, gate_proj, up_proj, down_proj)
    res = run_bass_kernel_spmd(nc, in_maps, core_ids=list(range(N_CORES)),
                               trace=TRACE)
    LAST_RESULTS = res

    expert_outs = np.stack([res.results[k]["out"] for k in range(N_CORES)])
    idx = np.asarray(expert_indices).astype(np.int64)  # [T, TOP_K]
    return expert_outs[idx, np.arange(T)[:, None], :].astype(np.float32)


# revision 20
# speedup vs baseline: 1.7537x; 1.0230x over previous
"""Expert-parallel MoE conditional feed-forward for 8 Trainium2 NeuronCores.

Problem: x[16,1024], expert_indices[16,2], gate/down_proj[8,2816,1024],
up_proj[8,1024,2816]. Reference computes, per (token, slot) pair with
e = expert_indices[t, a]:
    out[t,a,:] = (silu(x @ gate_proj[e].T) * (x @ down_proj[e].T)) @ up_proj[e].T

Sharding: core k owns expert k and computes its FFN output for ALL 16
tokens (the compute is negligible; the kernel is weight-streaming bound).
The host then gathers rows per expert_indices. This needs no indices on
device and is load-balanced regardless of routing.

Weights and x are cast to fp16 on the host (harness gate is 2e-2; fp16
end-to-end measures 4.7e-4 while fp8 e4m3 is mantissa-limited at 2.7e-2+
per matrix). 17.3 MB per core streams at the ~420 GB/s per-core fabric
ceiling measured when all 8 cores stream (~42 us).

Timeline anatomy (measured): exec_time = [first GpSimd MEMSET ... last
epilogue NOTIFY]. A ~5.8 us start rendezvous is excluded; a fixed
~8.4 us walrus epilogue (per-semaphore zeroing avalanche) is included.
So the controllable part is first-DMA-issue -> out-DMA-complete.

Key structural choices vs the previous revision:
  * Weight chunk 0's first DMA is issued BEFORE xt/eye so streaming (the
    critical path) starts ~1.3 us earlier.
  * Chunks are processed singly (not in pairs): chunk c's 4 up-matmuls
    are deferred only into chunk c+1's gate/down chain. That keeps the
    end-of-stream backlog to one chunk.
  * Chunks 0, 9, 10 stream as split DMAs; Tile's range-level dependency
    tracking lets consuming matmuls start as soon as their slice lands.
    Chunk 10 additionally uses a per-half column layout and is processed
    as two independent 128-wide half-chunks, so the dependency chain
    behind the very last weight byte is one 8-matmul N=256 chain + one
    silu/mul/transpose + 2 up-matmuls.
  * The final PSUM->SBUF copies run in parallel (jb0 on DVE, jb1 on the
    Scalar engine as an activation-Copy) before a single output DMA.

PE scheduling (array packing via PSUM base partition; q3 unusable per
HW bug): q1 (psum rows 32-47) and q2 (rows 64-79) alternate per chunk
for the gate|down chains; q0 (rows 0-15) carries all up-projection
accumulation into psum_out. The [16,128] fp16 intermediates are
transposed to [128,16] via identity matmuls on the PE, cast to fp16 on
the PSUM->SBUF copy, and fed as stationaries.
"""

import sys

for _p in ("/opt/trn_rl_repo", "/opt/pypackages"):
    if _p not in sys.path:
        sys.path.append(_p)

import numpy as np

NUM_EXPERTS = 8
HIDDEN = 1024
INTER = 2816
T = 16
N_CORES = 8
P = 128
PG = 120                  # gate/down weights use partitions 0..119 only.
                          # DMA descriptors are dealt to SDMA engines as
                          # equal contiguous partition blocks (largest
                          # divisor of the partition count <= 16), and an
                          # engine only hits full rate when its block spans
                          # >= 8 partitions (2 AXI ports). [120, X] DMAs ->
                          # 15 engines x 8 rows at full rate, with engine
                          # 15 (the chronically contended one) carrying no
                          # gate/down bytes at all. Hidden rows 960..1023
                          # go to a shared "leftover" tile.
CW = 256                  # intermediate chunk width
NCHUNK = INTER // CW      # 11
NFULL = NCHUNK - 1        # chunks 0..9 use the full-chunk layout
HC = HIDDEN // P          # 8 strips per gate/down chain (of PG rows each)
U_OFF = 2 * HC * CW       # 4096: offset of up blocks in packed W
WCOLS = U_OFF + 2 * HIDDEN  # 6144
N_UP = 2 * NFULL + 2      # 22 up-matmuls per 512-col output half

_COMPILED = None
LAST_RESULTS = None
TRACE = False


def _build():
    import concourse.bacc as bacc
    import concourse.bass as bass
    import concourse.tile as tile
    from concourse import mybir

    f32 = mybir.dt.float32
    f16 = mybir.dt.float16
    nc = bacc.Bacc("TRN2", target_bir_lowering=False, debug=False,
                   num_devices=N_CORES)
    xt_d = nc.dram_tensor("xt", [PG, HC * T], f16, kind="ExternalInput")
    xtl_d = nc.dram_tensor("xtl", [P, T], f16, kind="ExternalInput")
    eye_d = nc.dram_tensor("eye", [T, T], f16, kind="ExternalInput")
    lt_d = nc.dram_tensor("lt", [P, 6 * 512], f16, kind="ExternalInput")
    # Every DMA's DRAM source is its own contiguous block: column-sliced
    # reads of a wide row-major tensor interleave the HBM read stream at
    # stride 12 KB and measurably halve per-engine DMA throughput.
    wgdf_d = nc.dram_tensor("wgdf", [NFULL - 1, PG, U_OFF], f16,
                            kind="ExternalInput")
    wgds_d = nc.dram_tensor("wgds", [2, 2, PG, U_OFF // 2], f16,
                            kind="ExternalInput")
    wup_d = nc.dram_tensor("wup", [NFULL, P, 2 * HIDDEN], f16,
                           kind="ExternalInput")
    wup10_d = nc.dram_tensor("wup10", [2, P, HIDDEN], f16,
                             kind="ExternalInput")
    out_d = nc.dram_tensor("out", [T, HIDDEN], f32, kind="ExternalOutput")

    with tile.TileContext(nc) as tc:
        with (
            tc.tile_pool(name="xp", bufs=1) as xp,
            tc.tile_pool(name="wp", bufs=1) as wp,
            tc.tile_pool(name="ip", bufs=4) as ip,
            tc.tile_pool(name="itp", bufs=1) as itp,
            tc.tile_pool(name="pg", bufs=3, space=bass.MemorySpace.PSUM) as pgp,
            tc.tile_pool(name="tp", bufs=2, space=bass.MemorySpace.PSUM) as tpp,
            tc.tile_pool(name="po", bufs=1, space=bass.MemorySpace.PSUM) as pop,
            tc.tile_pool(name="op", bufs=1) as op,
        ):
            xt = xp.tile([PG, HC * T], f16)
            xtl = xp.tile([P, T], f16)
            eye = xp.tile([T, T], f16)
            lt = xp.tile([P, 6 * 512], f16)
            wt = [wp.tile([P, WCOLS], f16, name=f"w{c}", tag=f"w{c}")
                  for c in range(NCHUNK)]

            # xt/xtl/eye go via GpSimd (SWDGE) so the Sync engine issues
            # only weight DMAs back-to-back (descriptor generation is
            # ~650 ns per dma_start; keeping Sync weight-only shortens the
            # stream ramp). Gate/down DMAs move partitions 0..123 only;
            # the leftover tile lt carries hidden rows 992..1023 for all
            # chunks and lands early. Chunks 0, 9, 10 are split so
            # consuming matmuls start per-slice (Tile tracks range-level
            # DMA->reader deps).
            nc.gpsimd.dma_start(xt[:], xt_d.ap())
            nc.gpsimd.dma_start(xtl[:], xtl_d.ap())
            nc.gpsimd.dma_start(eye[:], eye_d.ap())

            H2 = U_OFF // 2
            nc.sync.dma_start(wt[0][0:PG, 0:H2], wgds_d.ap()[0][0])
            nc.sync.dma_start(lt[:], lt_d.ap())
            nc.sync.dma_start(wt[0][0:PG, H2:U_OFF], wgds_d.ap()[0][1])
            nc.sync.dma_start(wt[0][:, U_OFF:WCOLS], wup_d.ap()[0])
            for c in range(1, NFULL):
                nc.sync.dma_start(wt[c][0:PG, 0:U_OFF], wgdf_d.ap()[c - 1])
                nc.sync.dma_start(wt[c][:, U_OFF:WCOLS], wup_d.ap()[c])
            # chunk 10 (split layout): half h gd at [h*2048,(h+1)*2048),
            # half h up at [4096+h*1024, 4096+(h+1)*1024)
            nc.sync.dma_start(wt[10][0:PG, 0:H2], wgds_d.ap()[1][0])
            nc.sync.dma_start(wt[10][:, U_OFF:U_OFF + HIDDEN],
                              wup10_d.ap()[0])
            nc.sync.dma_start(wt[10][0:PG, H2:U_OFF], wgds_d.ap()[1][1])
            nc.sync.dma_start(wt[10][:, U_OFF + HIDDEN:WCOLS],
                              wup10_d.ap()[1])

            psum_out = pop.tile([T, HIDDEN], f32)
            itall = itp.tile([P, N_UP * T], f16)
            up_count = [0, 0]    # per-jb position in the accumulation chain
            pending = []         # PE thunks deferred from the previous chunk

            # All PE work downstream of a chunk's silu/mul (the transpose
            # and the two-or-four up-matmuls) is deferred into the NEXT
            # chunk's gate/down chain. The PE is strictly in-order, so
            # emitting a transpose right after its own chunk's chain would
            # stall the PE queue ~1.2 us waiting on ACT/DVE; one chunk
            # later the operands are long ready.
            def make_transpose(inter_slice, kidx):
                def t():
                    tp = tpp.tile([P, T], f32, name="tp")
                    nc.tensor.matmul(tp[:], inter_slice, eye[:])
                    nc.vector.tensor_copy(itall[:, kidx * T:(kidx + 1) * T],
                                          tp[:])
                return t

            def make_up(c, kidx, upbase, jb):
                def u():
                    k = up_count[jb]
                    up_count[jb] += 1
                    nc.tensor.matmul(
                        psum_out[:, jb * 512:(jb + 1) * 512],
                        itall[:, kidx * T:(kidx + 1) * T],
                        wt[c][:, upbase + jb * 512:upbase + (jb + 1) * 512],
                        start=(k == 0), stop=(k == N_UP - 1),
                    )
                return u

            def gd_chain(cols_of_hc, base, todo, m, ltlo, first_pop=0):
                # 8 strips of PG=120 hidden rows from the chunk's own tile,
                # then the 64-row leftover strip (hidden 960..1023) from
                # the shared lt tile at partition offset 64*m.
                pgd = pgp.tile([P, 2 * CW], f32, name="pgd")
                for hc in range(HC):
                    lo, width = cols_of_hc(hc)
                    nc.tensor.matmul(
                        pgd[base:base + T, 0:width],
                        xt[:, hc * T:(hc + 1) * T],
                        wt_cur[0:PG, lo:lo + width],
                        start=(hc == 0), stop=False,
                    )
                    if todo and hc >= first_pop:
                        todo.pop(0)()
                lo, width = cols_of_hc(0)
                nc.tensor.matmul(
                    pgd[base:base + T, 0:width],
                    xtl[64 * m:64 * m + 64, :],
                    lt[64 * m:64 * m + 64, ltlo:ltlo + width],
                    start=False, stop=True,
                )
                while todo:
                    todo.pop(0)()
                return pgd

            def silu_mul(pgd, base, width):
                s1 = ip.tile([T, width], f32, name="s1")
                nc.scalar.activation(s1[:], pgd[base:base + T, 0:width],
                                     mybir.ActivationFunctionType.Silu)
                inter = ip.tile([T, width], f16, name="inter")
                nc.vector.tensor_mul(inter[:], s1[:],
                                     pgd[base:base + T, width:2 * width])
                return inter

            for c in range(NFULL):
                base = 32 if c % 2 == 0 else 64
                wt_cur = wt[c]
                todo, pending = pending, []
                pgd = gd_chain(lambda hc: (hc * 2 * CW, 2 * CW), base, todo,
                               m=c % 2, ltlo=512 * (c // 2))
                inter = silu_mul(pgd, base, CW)
                for f in range(CW // P):
                    kidx = 2 * c + f
                    pending.append(
                        make_transpose(inter[:, f * P:(f + 1) * P], kidx))
                    pending.append(make_up(c, kidx, U_OFF + f * HIDDEN, 0))
                    pending.append(make_up(c, kidx, U_OFF + f * HIDDEN, 1))

            # chunk 10: two independent 128-wide halves so the chain behind
            # the last weight byte is as short as possible.
            wt_cur = wt[10]
            inter_h = []
            for h in range(2):
                base = 32 if h == 0 else 64
                todo, pending = pending, []
                # h1 pops its 3 deferred ops late (hc>=5): h0's mul is only
                # ~0.9 us behind, and popping early would stall the chain.
                pgd = gd_chain(
                    lambda hc, h=h: (h * 2048 + hc * CW, CW), base, todo,
                    m=0, ltlo=5 * 512 + h * CW,
                    first_pop=(0 if h == 0 else 5))
                inter_h.append(silu_mul(pgd, base, P))
                kidx = 2 * NFULL + h
                if h == 0:
                    pending.append(make_transpose(inter_h[0][:], kidx))
                    pending.append(make_up(10, kidx, U_OFF, 0))
                    pending.append(make_up(10, kidx, U_OFF, 1))
            # final strip: emit inline, jb0 then jb1, so the jb0 copy (DVE)
            # overlaps the jb1 matmul.
            kidx = 2 * NFULL + 1
            make_transpose(inter_h[1][:], kidx)()
            make_up(10, kidx, U_OFF + HIDDEN, 0)()
            make_up(10, kidx, U_OFF + HIDDEN, 1)()
            assert not pending
            assert up_count == [N_UP, N_UP], up_count

            # Final PSUM->SBUF copies run in parallel on DVE (jb0) and the
            # Scalar engine (jb1); each half then goes out via its own
            # HWDGE DMA (Sync and Scalar respectively) so the two output
            # transfers and their ~1.5 us completion latencies overlap.
            out_sb = op.tile([T, HIDDEN], f32)
            nc.vector.tensor_copy(out_sb[:, 0:512], psum_out[:, 0:512])
            nc.sync.dma_start(out_d.ap()[:, 0:512], out_sb[:, 0:512])
            nc.scalar.activation(out_sb[:, 512:1024], psum_out[:, 512:1024],
                                 mybir.ActivationFunctionType.Copy)
            nc.scalar.dma_start(out_d.ap()[:, 512:1024],
                                out_sb[:, 512:1024])

    nc.compile()
    return nc


def _get_compiled():
    global _COMPILED
    if _COMPILED is None:
        _COMPILED = _build()
    return _COMPILED


def _pack_inputs(x, gate_proj, up_proj, down_proj):
    HM = HC * PG  # 992 hidden rows on the main 124-partition strips
    x = np.ascontiguousarray(x, dtype=np.float32)
    # xt[p, s*T + t] = x[t, s*124 + p]
    xt = np.ascontiguousarray(
        x[:, 0:HM].T.reshape(HC, PG, T).transpose(1, 0, 2).reshape(PG, HC * T)
    ).astype(np.float16)
    # xtl[64m + j, t] = x[t, 960 + j], replicated for m=0,1 so the
    # leftover matmul's stationary matches either 64m partition offset.
    xtl = np.ascontiguousarray(
        np.tile(x[:, HM:HIDDEN].T, (2, 1))).astype(np.float16)
    eye = np.eye(T, dtype=np.float16)
    in_maps = []
    for k in range(N_CORES):
        g = np.asarray(gate_proj[k], dtype=np.float32)
        d = np.asarray(down_proj[k], dtype=np.float32)
        u = np.asarray(up_proj[k], dtype=np.float32)
        w = np.zeros((NCHUNK, P, WCOLS), dtype=np.float16)
        # Full chunks 0..9 gd: w[c, p, s*512 + o] = g/d[c*CW + o', s*124+p]
        # (strips s of 124 hidden rows; [g 256 | d 256] interleave per s).
        gm = g[:, 0:HM].reshape(NCHUNK, CW, HC, PG).transpose(0, 3, 2, 1)
        dm = d[:, 0:HM].reshape(NCHUNK, CW, HC, PG).transpose(0, 3, 2, 1)
        gdm = np.concatenate([gm, dm], axis=3).reshape(NCHUNK, PG, U_OFF)
        w[:, 0:PG, 0:U_OFF] = gdm
        # Chunk 10 split gd layout: w[10, p, h*2048 + s*256 + o]:
        #   o<128: g[10*CW + h*128 + o, s*124+p]; o>=128: d[.. o-128 ..]
        c = NCHUNK - 1
        gl = g[c * CW:, 0:HM].reshape(2, P, HC, PG).transpose(0, 2, 3, 1)
        dl = d[c * CW:, 0:HM].reshape(2, P, HC, PG).transpose(0, 2, 3, 1)
        gdl = np.concatenate([gl, dl], axis=3)  # [2, HC, PG, 256]
        w[c, 0:PG, 0:U_OFF] = gdl.transpose(2, 0, 1, 3).reshape(PG, U_OFF)
        # Up blocks (all chunks): w[c, p, U_OFF + f*HIDDEN + j] =
        #   u[j, c*CW + f*128 + p]
        w[:, :, U_OFF:] = u.reshape(HIDDEN, NCHUNK, CW // P, P).transpose(
            1, 3, 2, 0).reshape(NCHUNK, P, 2 * HIDDEN)
        # Leftover tile: hidden rows 960..1023 for every chunk. Chunk
        # c = 2b + m sits at partitions [64m, 64m+64), cols [512b, 512b+512)
        # with the same [g|d] interleave as its chain ([g256|d256] for full
        # chunks; [g128|d128] per half for chunk 10 at b=5, m=0: half h at
        # cols [2560 + h*256, ...)).
        ltile = np.zeros((P, 6 * 512), dtype=np.float16)
        gr = g[:, HM:HIDDEN]  # [INTER, 64]
        dr = d[:, HM:HIDDEN]
        for c2 in range(NFULL):
            b, m = divmod(c2, 2)
            blk = np.concatenate(
                [gr[c2 * CW:(c2 + 1) * CW], dr[c2 * CW:(c2 + 1) * CW]],
                axis=0)  # [512, 64]
            ltile[64 * m:64 * m + 64, 512 * b:512 * b + 512] = blk.T
        for h in range(2):
            lo = c * CW + h * P
            blk = np.concatenate([gr[lo:lo + P], dr[lo:lo + P]], axis=0)
            ltile[0:64, 5 * 512 + h * CW:5 * 512 + (h + 1) * CW] = blk.T
        # Contiguous per-DMA source blocks (see dram_tensor comment).
        wgdf = np.ascontiguousarray(w[1:NFULL, 0:PG, 0:U_OFF])
        wgds = np.ascontiguousarray(np.stack([
            w[0, 0:PG, 0:U_OFF].reshape(PG, 2, U_OFF // 2).transpose(1, 0, 2),
            w[c, 0:PG, 0:U_OFF].reshape(PG, 2, U_OFF // 2).transpose(1, 0, 2),
        ]))
        wup = np.ascontiguousarray(w[0:NFULL, :, U_OFF:])
        wup10 = np.ascontiguousarray(
            w[c, :, U_OFF:].reshape(P, 2, HIDDEN).transpose(1, 0, 2))
        in_maps.append({"xt": xt, "xtl": xtl, "eye": eye, "lt": ltile,
                        "wgdf": wgdf, "wgds": wgds, "wup": wup,
                        "wup10": wup10})
    return in_maps


def kernel(x, expert_indices, gate_proj, up_proj, down_proj):
    global LAST_RESULTS
    from concourse.bass_utils import run_bass_kernel_spmd

    nc = _get_compiled()
    in_maps = _pack_inputs(x, gate_proj, up_proj, down_proj)
    res = run_bass_kernel_spmd(nc, in_maps, core_ids=list(range(N_CORES)),
                               trace=TRACE)
    LAST_RESULTS = res

    expert_outs = np.stack([res.results[k]["out"] for k in range(N_CORES)])
    idx = np.asarray(expert_indices).astype(np.int64)  # [T, TOP_K]
    return expert_outs[idx, np.arange(T)[:, None], :].astype(np.float32)


# revision 21
# speedup vs baseline: 2.2592x; 1.2882x over previous
"""Expert-parallel MoE conditional feed-forward for 8 Trainium2 NeuronCores.

Problem: x[16,1024], expert_indices[16,2], gate/down_proj[8,2816,1024],
up_proj[8,1024,2816]. Reference computes, per (token, slot) pair with
e = expert_indices[t, a]:
    out[t,a,:] = (silu(x @ gate_proj[e].T) * (x @ down_proj[e].T)) @ up_proj[e].T

Sharding: core k owns expert k and computes its FFN output for ALL 16
tokens (the compute is negligible; the kernel is weight-streaming bound).
The host then gathers rows per expert_indices. This needs no indices on
device and is load-balanced regardless of routing.

Weights and x are cast to fp16 on the host (harness gate is 2e-2; fp16
end-to-end measures 4.7e-4 while fp8 e4m3 is mantissa-limited at 2.7e-2+
per matrix). 17.3 MB per core streams at the ~420 GB/s per-core ceiling
measured when all 8 cores stream (~42-44 us).

Measured timeline anatomy: exec_time = [first GpSimd MEMSET ... last
epilogue NOTIFY]. A ~5.8 us start rendezvous is excluded; a fixed
~8.4 us walrus epilogue (per-semaphore zeroing avalanche) is included.
The controllable part is first-DMA-issue -> out-DMA-complete.

Key structural choices (each measured on HW):
  * Weight DMAs keep the per-chunk [128, 12288B] single-DMA shape: the
    descriptor dealer assigns equal contiguous partition blocks to the 16
    SDMA engines, and this shape is the only one measured to sustain
    ~26 B/ns per engine. Splitting chunks into [120/124, X] row-sliced
    DMAs (tried, to starve the chronically-contended SDMA engine 15)
    deals correctly but halves per-engine throughput - reverted.
  * Chunk 0's first piece is issued before xt/eye (which go via GpSimd /
    SWDGE) so weight streaming starts ~1.3 us earlier and the Sync
    engine issues weight DMAs back-to-back.
  * Chunks are processed singly; all PE work downstream of a chunk's
    silu/mul (transpose + up-matmuls) is deferred into the NEXT chunk's
    gate/down chain. The PE is strictly in-order, so emitting a
    transpose right after its own chunk's chain would stall the PE queue
    ~1.2 us waiting on ACT/DVE; one chunk later the operands are ready.
  * Chunks 0, 9, 10 stream as split DMAs; Tile's range-level dependency
    tracking lets consuming matmuls start as soon as their slice lands.
    Chunk 10 uses a per-half column layout and is processed as two
    independent 128-wide half-chunks, so the chain behind the very last
    weight byte is one 8-matmul N=256 chain + silu/mul/transpose + 2
    up-matmuls.
  * The final PSUM->SBUF copies run in parallel (jb0 on DVE, jb1 on the
    Scalar engine as an activation-Copy) and each 512-col half goes out
    via its own HWDGE DMA (Sync and Scalar respectively) so the two
    ~1.5 us output completion latencies overlap.

PE scheduling (array packing via PSUM base partition; q3 unusable per
HW bug): q1 (psum rows 32-47) and q2 (rows 64-79) alternate per chunk
for the gate|down chains; q0 (rows 0-15) carries all up-projection
accumulation into psum_out. The [16,128] fp16 intermediates are
transposed to [128,16] via identity matmuls on the PE, cast to fp16 on
the PSUM->SBUF copy, and fed as stationaries.
"""

import sys

for _p in ("/opt/trn_rl_repo", "/opt/pypackages"):
    if _p not in sys.path:
        sys.path.append(_p)

import numpy as np

NUM_EXPERTS = 8
HIDDEN = 1024
INTER = 2816
T = 16
N_CORES = 8
P = 128
CW = 256                  # intermediate chunk width
NCHUNK = INTER // CW      # 11
NFULL = NCHUNK - 1        # chunks 0..9 use the full-chunk layout
HC = HIDDEN // P          # 8 hidden strips per gate/down chain
U_OFF = 2 * HC * CW       # 4096: offset of up blocks in packed W
WCOLS = U_OFF + 2 * HIDDEN  # 6144
N_UP = 2 * NFULL + 2      # 22 up-matmuls per 512-col output half

_COMPILED = None
LAST_RESULTS = None
TRACE = False


def _build():
    import concourse.bacc as bacc
    import concourse.bass as bass
    import concourse.tile as tile
    from concourse import mybir

    f32 = mybir.dt.float32
    f16 = mybir.dt.float16
    nc = bacc.Bacc("TRN2", target_bir_lowering=False, debug=False,
                   num_devices=N_CORES)
    xt_d = nc.dram_tensor("xt", [P, HC * T], f16, kind="ExternalInput")
    eye_d = nc.dram_tensor("eye", [T, T], f16, kind="ExternalInput")
    w_d = nc.dram_tensor("w", [NCHUNK, P, WCOLS], f16, kind="ExternalInput")
    out_d = nc.dram_tensor("out", [T, HIDDEN], f32, kind="ExternalOutput")

    with tile.TileContext(nc) as tc:
        with (
            tc.tile_pool(name="xp", bufs=1) as xp,
            tc.tile_pool(name="wp", bufs=1) as wp,
            tc.tile_pool(name="ip", bufs=4) as ip,
            tc.tile_pool(name="itp", bufs=1) as itp,
            tc.tile_pool(name="pg", bufs=3, space=bass.MemorySpace.PSUM) as pgp,
            tc.tile_pool(name="tp", bufs=2, space=bass.MemorySpace.PSUM) as tpp,
            tc.tile_pool(name="po", bufs=1, space=bass.MemorySpace.PSUM) as pop,
            tc.tile_pool(name="op", bufs=1) as op,
        ):
            xt = xp.tile([P, HC * T], f16)
            eye = xp.tile([T, T], f16)
            wt = [wp.tile([P, WCOLS], f16, name=f"w{c}", tag=f"w{c}")
                  for c in range(NCHUNK)]

            # xt/eye go via GpSimd (SWDGE) so the Sync engine issues only
            # weight DMAs back-to-back.
            nc.gpsimd.dma_start(xt[:], xt_d.ap())
            nc.gpsimd.dma_start(eye[:], eye_d.ap())
            nc.sync.dma_start(wt[0][:, 0:U_OFF // 2],
                              w_d.ap()[0][:, 0:U_OFF // 2])
            nc.sync.dma_start(wt[0][:, U_OFF // 2:U_OFF],
                              w_d.ap()[0][:, U_OFF // 2:U_OFF])
            nc.sync.dma_start(wt[0][:, U_OFF:WCOLS],
                              w_d.ap()[0][:, U_OFF:WCOLS])
            for c in range(1, NFULL - 1):
                nc.sync.dma_start(wt[c][:], w_d.ap()[c])
            c = NFULL - 1  # chunk 9: gate/down then up
            nc.sync.dma_start(wt[c][:, 0:U_OFF], w_d.ap()[c][:, 0:U_OFF])
            nc.sync.dma_start(wt[c][:, U_OFF:WCOLS],
                              w_d.ap()[c][:, U_OFF:WCOLS])
            # chunk 10 (split layout): half h gd at [h*2048,(h+1)*2048),
            # half h up at [4096+h*1024, 4096+(h+1)*1024)
            nc.sync.dma_start(wt[10][:, 0:2048], w_d.ap()[10][:, 0:2048])
            nc.sync.dma_start(wt[10][:, U_OFF:U_OFF + HIDDEN],
                              w_d.ap()[10][:, U_OFF:U_OFF + HIDDEN])
            nc.sync.dma_start(wt[10][:, 2048:U_OFF],
                              w_d.ap()[10][:, 2048:U_OFF])
            nc.sync.dma_start(wt[10][:, U_OFF + HIDDEN:WCOLS],
                              w_d.ap()[10][:, U_OFF + HIDDEN:WCOLS])

            psum_out = pop.tile([T, HIDDEN], f32)
            itall = itp.tile([P, N_UP * T], f16)
            up_count = [0, 0]    # per-jb position in the accumulation chain
            pending = []         # PE thunks deferred from the previous chunk

            def make_transpose(inter_slice, kidx):
                def t():
                    tp = tpp.tile([P, T], f32, name="tp")
                    nc.tensor.matmul(tp[:], inter_slice, eye[:])
                    nc.vector.tensor_copy(itall[:, kidx * T:(kidx + 1) * T],
                                          tp[:])
                return t

            def make_up(c, kidx, upbase, jb):
                def u():
                    k = up_count[jb]
                    up_count[jb] += 1
                    nc.tensor.matmul(
                        psum_out[:, jb * 512:(jb + 1) * 512],
                        itall[:, kidx * T:(kidx + 1) * T],
                        wt[c][:, upbase + jb * 512:upbase + (jb + 1) * 512],
                        start=(k == 0), stop=(k == N_UP - 1),
                    )
                return u

            def gd_chain(cols_of_hc, base, todo, first_pop=0):
                pgd = pgp.tile([P, 2 * CW], f32, name="pgd")
                for hc in range(HC):
                    lo, width = cols_of_hc(hc)
                    nc.tensor.matmul(
                        pgd[base:base + T, 0:width],
                        xt[:, hc * T:(hc + 1) * T],
                        wt_cur[:, lo:lo + width],
                        start=(hc == 0), stop=(hc == HC - 1),
                    )
                    if todo and hc >= first_pop:
                        todo.pop(0)()
                while todo:
                    todo.pop(0)()
                return pgd

            def silu_mul(pgd, base, width):
                s1 = ip.tile([T, width], f32, name="s1")
                nc.scalar.activation(s1[:], pgd[base:base + T, 0:width],
                                     mybir.ActivationFunctionType.Silu)
                inter = ip.tile([T, width], f16, name="inter")
                nc.vector.tensor_mul(inter[:], s1[:],
                                     pgd[base:base + T, width:2 * width])
                return inter

            for c in range(NFULL):
                base = 32 if c % 2 == 0 else 64
                wt_cur = wt[c]
                todo, pending = pending, []
                pgd = gd_chain(lambda hc: (hc * 2 * CW, 2 * CW), base, todo)
                inter = silu_mul(pgd, base, CW)
                for f in range(CW // P):
                    kidx = 2 * c + f
                    pending.append(
                        make_transpose(inter[:, f * P:(f + 1) * P], kidx))
                    pending.append(make_up(c, kidx, U_OFF + f * HIDDEN, 0))
                    pending.append(make_up(c, kidx, U_OFF + f * HIDDEN, 1))

            # chunk 10: two independent 128-wide halves so the chain behind
            # the last weight byte is as short as possible.
            wt_cur = wt[10]
            inter_h = []
            for h in range(2):
                base = 32 if h == 0 else 64
                todo, pending = pending, []
                # h1 pops its 3 deferred ops late (hc>=5): h0's mul is only
                # ~0.9 us behind, and popping early would stall the chain.
                pgd = gd_chain(
                    lambda hc, h=h: (h * 2048 + hc * CW, CW), base, todo,
                    first_pop=(0 if h == 0 else 5))
                inter_h.append(silu_mul(pgd, base, P))
                kidx = 2 * NFULL + h
                if h == 0:
                    pending.append(make_transpose(inter_h[0][:], kidx))
                    pending.append(make_up(10, kidx, U_OFF, 0))
                    pending.append(make_up(10, kidx, U_OFF, 1))
            # final strip: emit inline, jb0 then jb1, so the jb0 copy (DVE)
            # overlaps the jb1 matmul.
            kidx = 2 * NFULL + 1
            make_transpose(inter_h[1][:], kidx)()
            make_up(10, kidx, U_OFF + HIDDEN, 0)()
            make_up(10, kidx, U_OFF + HIDDEN, 1)()
            assert not pending
            assert up_count == [N_UP, N_UP], up_count

            # Final PSUM->SBUF copies run in parallel on DVE (jb0) and the
            # Scalar engine (jb1); each half then goes out via its own
            # HWDGE DMA (Sync and Scalar respectively) so the two output
            # transfers and their ~1.5 us completion latencies overlap.
            out_sb = op.tile([T, HIDDEN], f32)
            nc.vector.tensor_copy(out_sb[:, 0:512], psum_out[:, 0:512])
            nc.sync.dma_start(out_d.ap()[:, 0:512], out_sb[:, 0:512])
            nc.scalar.activation(out_sb[:, 512:1024], psum_out[:, 512:1024],
                                 mybir.ActivationFunctionType.Copy)
            nc.scalar.dma_start(out_d.ap()[:, 512:1024],
                                out_sb[:, 512:1024])

    nc.compile()
    return nc


def _get_compiled():
    global _COMPILED
    if _COMPILED is None:
        _COMPILED = _build()
    return _COMPILED


def _pack_inputs(x, gate_proj, up_proj, down_proj):
    x = np.ascontiguousarray(x, dtype=np.float32)
    # xt[p, hc*T + t] = x[t, hc*128 + p]
    xt = np.ascontiguousarray(
        x.T.reshape(HC, P, T).transpose(1, 0, 2).reshape(P, HC * T)
    ).astype(np.float16)
    eye = np.eye(T, dtype=np.float16)
    in_maps = []
    for k in range(N_CORES):
        g = np.asarray(gate_proj[k], dtype=np.float32)
        d = np.asarray(down_proj[k], dtype=np.float32)
        u = np.asarray(up_proj[k], dtype=np.float32)
        # Full chunks 0..9: wg4/wd4[c, p, hc, o] = g/d[c*CW+o, hc*128+p],
        # interleaved [g 256 | d 256] per hc block.
        wg4 = g.reshape(NCHUNK, CW, HC, P).transpose(0, 3, 2, 1)
        wd4 = d.reshape(NCHUNK, CW, HC, P).transpose(0, 3, 2, 1)
        wgd = np.concatenate([wg4, wd4], axis=3).reshape(NCHUNK, P, 2 * HC * CW)
        # Wu[c, p, f*HIDDEN + j] = u[j, c*CW + f*128 + p]
        wu = u.reshape(HIDDEN, NCHUNK, CW // P, P).transpose(1, 3, 2, 0).reshape(
            NCHUNK, P, 2 * HIDDEN)
        w = np.concatenate([wgd, wu], axis=2).astype(np.float16)
        # Chunk 10 split gd layout: half h at cols [h*2048 + hc*256 + o]:
        #   o<128: g[10*CW + h*128 + o, hc*128+p]
        #   o>=128: d[10*CW + h*128 + (o-128), hc*128+p]
        # (up layout for chunk 10 is identical to the full-chunk layout.)
        c = NCHUNK - 1
        glast = g[c * CW:(c + 1) * CW].reshape(2, P, HC, P).transpose(0, 2, 3, 1)
        dlast = d[c * CW:(c + 1) * CW].reshape(2, P, HC, P).transpose(0, 2, 3, 1)
        gdlast = np.concatenate([glast, dlast], axis=3)  # [2, HC, P, 256]
        w[c, :, 0:U_OFF] = gdlast.transpose(2, 0, 1, 3).reshape(P, U_OFF)
        in_maps.append({"xt": xt, "eye": eye,
                        "w": np.ascontiguousarray(w)})
    return in_maps


def kernel(x, expert_indices, gate_proj, up_proj, down_proj):
    global LAST_RESULTS
    from concourse.bass_utils import run_bass_kernel_spmd

    nc = _get_compiled()
    in_maps = _pack_inputs(x, gate_proj, up_proj, down_proj)
    res = run_bass_kernel_spmd(nc, in_maps, core_ids=list(range(N_CORES)),
                               trace=TRACE)
    LAST_RESULTS = res

    expert_outs = np.stack([res.results[k]["out"] for k in range(N_CORES)])
    idx = np.asarray(expert_indices).astype(np.int64)  # [T, TOP_K]
    return expert_outs[idx, np.arange(T)[:, None], :].astype(np.float32)


# revision 23
# speedup vs baseline: 2.5489x; 1.1283x over previous
"""Expert-parallel MoE conditional feed-forward for 8 Trainium2 NeuronCores.

Problem: x[16,1024], expert_indices[16,2], gate/down_proj[8,2816,1024],
up_proj[8,1024,2816]. Reference computes, per (token, slot) pair with
e = expert_indices[t, a]:
    out[t,a,:] = (silu(x @ gate_proj[e].T) * (x @ down_proj[e].T)) @ up_proj[e].T

Sharding: core k owns expert k and computes its FFN output for ALL 16
tokens (the compute is negligible; the kernel is weight-streaming bound).
The host then gathers rows per expert_indices. This needs no indices on
device and is load-balanced regardless of routing.

Weights and x are cast to fp16 on the host (harness gate is 2e-2; fp16
end-to-end measures 4.7e-4 while fp8 e4m3 is mantissa-limited at 2.7e-2+
per matrix). 17.3 MB per core streams at the ~420 GB/s per-core ceiling
measured when all 8 cores stream (~42-44 us).

Measured timeline anatomy: exec_time = [first GpSimd MEMSET ... last
epilogue NOTIFY]. A ~5.8 us start rendezvous is excluded; a fixed
~8.4 us walrus epilogue (per-semaphore zeroing avalanche) is included.
The controllable part is first-DMA-issue -> out-DMA-complete.

Key structural choices (each measured on HW):
  * Weight DMAs keep the per-chunk [128, 12288B] single-DMA shape: the
    descriptor dealer assigns equal contiguous partition blocks to the 16
    SDMA engines, and this shape is the only one measured to sustain
    ~26 B/ns per engine. Splitting chunks into [120/124, X] row-sliced
    DMAs (tried, to starve the chronically-contended SDMA engine 15)
    deals correctly but halves per-engine throughput - reverted.
  * Chunk 0's first piece is issued before xt/eye (which go via GpSimd /
    SWDGE) so weight streaming starts ~1.3 us earlier and the Sync
    engine issues weight DMAs back-to-back.
  * Chunks are processed singly; all PE work downstream of a chunk's
    silu/mul (transpose + up-matmuls) is deferred into the NEXT chunk's
    gate/down chain. The PE is strictly in-order, so emitting a
    transpose right after its own chunk's chain would stall the PE queue
    ~1.2 us waiting on ACT/DVE; one chunk later the operands are ready.
  * Chunks 0, 9, 10 stream as split DMAs; Tile's range-level dependency
    tracking lets consuming matmuls start as soon as their slice lands.
    Chunk 10 uses a per-half column layout and is processed as two
    independent 128-wide half-chunks, so the chain behind the very last
    weight byte is one 8-matmul N=256 chain + silu/mul/transpose + 2
    up-matmuls.
  * The final PSUM->SBUF copies run in parallel (jb0 on DVE, jb1 on the
    Scalar engine as an activation-Copy) and each 512-col half goes out
    via its own HWDGE DMA (Sync and Scalar respectively) so the two
    ~1.5 us output completion latencies overlap.

PE scheduling (array packing via PSUM base partition; q3 unusable per
HW bug): q1 (psum rows 32-47) and q2 (rows 64-79) alternate per chunk
for the gate|down chains; q0 (rows 0-15) carries all up-projection
accumulation into psum_out. The [16,128] fp16 intermediates are
transposed to [128,16] via identity matmuls on the PE, cast to fp16 on
the PSUM->SBUF copy, and fed as stationaries.
"""

import sys

for _p in ("/opt/trn_rl_repo", "/opt/pypackages"):
    if _p not in sys.path:
        sys.path.append(_p)

import numpy as np

NUM_EXPERTS = 8
HIDDEN = 1024
INTER = 2816
T = 16
N_CORES = 8
P = 128
CW = 256                  # intermediate chunk width
NCHUNK = INTER // CW      # 11
NFULL = NCHUNK - 1        # chunks 0..9 use the full-chunk layout
HC = HIDDEN // P          # 8 hidden strips per gate/down chain
U_OFF = 2 * HC * CW       # 4096: offset of up blocks in packed W
WCOLS = U_OFF + 2 * HIDDEN  # 6144
N_UP = 2 * NFULL + 2      # 22 up-matmuls per 512-col output half

_COMPILED = None
LAST_RESULTS = None
TRACE = False


def _build():
    import concourse.bacc as bacc
    import concourse.bass as bass
    import concourse.tile as tile
    from concourse import mybir

    f32 = mybir.dt.float32
    f16 = mybir.dt.float16
    nc = bacc.Bacc("TRN2", target_bir_lowering=False, debug=False,
                   num_devices=N_CORES)
    xt_d = nc.dram_tensor("xt", [P, HC * T], f16, kind="ExternalInput")
    eye_d = nc.dram_tensor("eye", [T, T], f16, kind="ExternalInput")
    w_d = nc.dram_tensor("w", [NCHUNK, P, WCOLS], f16, kind="ExternalInput")
    out_d = nc.dram_tensor("out", [T, HIDDEN], f32, kind="ExternalOutput")

    with tile.TileContext(nc) as tc:
        with (
            tc.tile_pool(name="xp", bufs=1) as xp,
            tc.tile_pool(name="wp", bufs=1) as wp,
            tc.tile_pool(name="ip", bufs=4) as ip,
            tc.tile_pool(name="itp", bufs=1) as itp,
            tc.tile_pool(name="pg", bufs=3, space=bass.MemorySpace.PSUM) as pgp,
            tc.tile_pool(name="tp", bufs=2, space=bass.MemorySpace.PSUM) as tpp,
            tc.tile_pool(name="po", bufs=1, space=bass.MemorySpace.PSUM) as pop,
            tc.tile_pool(name="op", bufs=1) as op,
        ):
            xt = xp.tile([P, HC * T], f16)
            eye = xp.tile([T, T], f16)
            wt = [wp.tile([P, WCOLS], f16, name=f"w{c}", tag=f"w{c}")
                  for c in range(NCHUNK)]

            # xt/eye go via GpSimd (SWDGE) so the Sync engine issues only
            # weight DMAs back-to-back.
            nc.gpsimd.dma_start(xt[:], xt_d.ap())
            nc.gpsimd.dma_start(eye[:], eye_d.ap())
            nc.sync.dma_start(wt[0][:, 0:U_OFF // 2],
                              w_d.ap()[0][:, 0:U_OFF // 2])
            nc.sync.dma_start(wt[0][:, U_OFF // 2:U_OFF],
                              w_d.ap()[0][:, U_OFF // 2:U_OFF])
            nc.sync.dma_start(wt[0][:, U_OFF:WCOLS],
                              w_d.ap()[0][:, U_OFF:WCOLS])
            for c in range(1, NFULL - 1):
                nc.sync.dma_start(wt[c][:], w_d.ap()[c])
            c = NFULL - 1  # chunk 9: gate/down then up
            nc.sync.dma_start(wt[c][:, 0:U_OFF], w_d.ap()[c][:, 0:U_OFF])
            nc.sync.dma_start(wt[c][:, U_OFF:WCOLS],
                              w_d.ap()[c][:, U_OFF:WCOLS])
            # chunk 10 (split layout): half h gd at [h*2048,(h+1)*2048),
            # half h up at [4096+h*1024, 4096+(h+1)*1024). Both up pieces
            # go BEFORE the gd pieces so the final up-matmuls never wait
            # on a DMA; gd-h1 is the last weight byte on the wire.
            nc.sync.dma_start(wt[10][:, U_OFF:U_OFF + HIDDEN],
                              w_d.ap()[10][:, U_OFF:U_OFF + HIDDEN])
            nc.sync.dma_start(wt[10][:, U_OFF + HIDDEN:WCOLS],
                              w_d.ap()[10][:, U_OFF + HIDDEN:WCOLS])
            nc.sync.dma_start(wt[10][:, 0:2048], w_d.ap()[10][:, 0:2048])
            nc.sync.dma_start(wt[10][:, 2048:U_OFF],
                              w_d.ap()[10][:, 2048:U_OFF])

            psum_out = pop.tile([T, HIDDEN], f32)
            itall = itp.tile([P, N_UP * T], f16)
            up_count = [0, 0]    # per-jb position in the accumulation chain
            pending = []         # PE thunks deferred from the previous chunk

            def make_transpose(inter_slice, kidx):
                def t():
                    tp = tpp.tile([P, T], f32, name="tp")
                    nc.tensor.matmul(tp[:], inter_slice, eye[:])
                    nc.vector.tensor_copy(itall[:, kidx * T:(kidx + 1) * T],
                                          tp[:])
                return t

            def make_up(c, kidx, upbase, jb):
                def u():
                    k = up_count[jb]
                    up_count[jb] += 1
                    nc.tensor.matmul(
                        psum_out[:, jb * 512:(jb + 1) * 512],
                        itall[:, kidx * T:(kidx + 1) * T],
                        wt[c][:, upbase + jb * 512:upbase + (jb + 1) * 512],
                        start=(k == 0), stop=(k == N_UP - 1),
                    )
                return u

            def gd_chain(cols_of_hc, base, todo, first_pop=0):
                pgd = pgp.tile([P, 2 * CW], f32, name="pgd")
                for hc in range(HC):
                    lo, width = cols_of_hc(hc)
                    nc.tensor.matmul(
                        pgd[base:base + T, 0:width],
                        xt[:, hc * T:(hc + 1) * T],
                        wt_cur[:, lo:lo + width],
                        start=(hc == 0), stop=(hc == HC - 1),
                    )
                    if todo and hc >= first_pop:
                        todo.pop(0)()
                while todo:
                    todo.pop(0)()
                return pgd

            def silu_mul(pgd, base, width):
                s1 = ip.tile([T, width], f32, name="s1")
                nc.scalar.activation(s1[:], pgd[base:base + T, 0:width],
                                     mybir.ActivationFunctionType.Silu)
                inter = ip.tile([T, width], f16, name="inter")
                nc.vector.tensor_mul(inter[:], s1[:],
                                     pgd[base:base + T, width:2 * width])
                return inter

            for c in range(NFULL):
                base = 32 if c % 2 == 0 else 64
                wt_cur = wt[c]
                todo, pending = pending, []
                pgd = gd_chain(lambda hc: (hc * 2 * CW, 2 * CW), base, todo)
                inter = silu_mul(pgd, base, CW)
                for f in range(CW // P):
                    kidx = 2 * c + f
                    pending.append(
                        make_transpose(inter[:, f * P:(f + 1) * P], kidx))
                    pending.append(make_up(c, kidx, U_OFF + f * HIDDEN, 0))
                    pending.append(make_up(c, kidx, U_OFF + f * HIDDEN, 1))

            # chunk 10: two independent 128-wide halves so the chain behind
            # the last weight byte is as short as possible.
            wt_cur = wt[10]
            inter_h = []
            for h in range(2):
                base = 32 if h == 0 else 64
                todo, pending = pending, []
                # h1 pops its 3 deferred ops late (hc>=6): h0's mul is only
                # ~0.9 us behind, and popping early would stall the chain.
                pgd = gd_chain(
                    lambda hc, h=h: (h * 2048 + hc * CW, CW), base, todo,
                    first_pop=(0 if h == 0 else 6))
                inter_h.append(silu_mul(pgd, base, P))
                kidx = 2 * NFULL + h
                if h == 0:
                    pending.append(make_transpose(inter_h[0][:], kidx))
                    pending.append(make_up(10, kidx, U_OFF, 0))
                    pending.append(make_up(10, kidx, U_OFF, 1))
            # final strip: emit inline, jb0 then jb1, so the jb0 copy (DVE)
            # overlaps the jb1 matmul.
            kidx = 2 * NFULL + 1
            make_transpose(inter_h[1][:], kidx)()
            make_up(10, kidx, U_OFF + HIDDEN, 0)()
            make_up(10, kidx, U_OFF + HIDDEN, 1)()
            assert not pending
            assert up_count == [N_UP, N_UP], up_count

            # Final PSUM->SBUF copies run in parallel on DVE (jb0) and the
            # Scalar engine (jb1); each half then goes out via its own
            # HWDGE DMA (Sync and Scalar respectively) so the two output
            # transfers and their ~1.5 us completion latencies overlap.
            out_sb = op.tile([T, HIDDEN], f32)
            nc.vector.tensor_copy(out_sb[:, 0:512], psum_out[:, 0:512])
            nc.sync.dma_start(out_d.ap()[:, 0:512], out_sb[:, 0:512])
            nc.scalar.activation(out_sb[:, 512:1024], psum_out[:, 512:1024],
                                 mybir.ActivationFunctionType.Copy)
            nc.scalar.dma_start(out_d.ap()[:, 512:1024],
                                out_sb[:, 512:1024])

    nc.compile()
    return nc


def _get_compiled():
    global _COMPILED
    if _COMPILED is None:
        _COMPILED = _build()
    return _COMPILED


def _pack_inputs(x, gate_proj, up_proj, down_proj):
    x = np.ascontiguousarray(x, dtype=np.float32)
    # xt[p, hc*T + t] = x[t, hc*128 + p]
    xt = np.ascontiguousarray(
        x.T.reshape(HC, P, T).transpose(1, 0, 2).reshape(P, HC * T)
    ).astype(np.float16)
    eye = np.eye(T, dtype=np.float16)
    in_maps = []
    for k in range(N_CORES):
        g = np.asarray(gate_proj[k], dtype=np.float32)
        d = np.asarray(down_proj[k], dtype=np.float32)
        u = np.asarray(up_proj[k], dtype=np.float32)
        # Full chunks 0..9: wg4/wd4[c, p, hc, o] = g/d[c*CW+o, hc*128+p],
        # interleaved [g 256 | d 256] per hc block.
        wg4 = g.reshape(NCHUNK, CW, HC, P).transpose(0, 3, 2, 1)
        wd4 = d.reshape(NCHUNK, CW, HC, P).transpose(0, 3, 2, 1)
        wgd = np.concatenate([wg4, wd4], axis=3).reshape(NCHUNK, P, 2 * HC * CW)
        # Wu[c, p, f*HIDDEN + j] = u[j, c*CW + f*128 + p]
        wu = u.reshape(HIDDEN, NCHUNK, CW // P, P).transpose(1, 3, 2, 0).reshape(
            NCHUNK, P, 2 * HIDDEN)
        w = np.concatenate([wgd, wu], axis=2).astype(np.float16)
        # Chunk 10 split gd layout: half h at cols [h*2048 + hc*256 + o]:
        #   o<128: g[10*CW + h*128 + o, hc*128+p]
        #   o>=128: d[10*CW + h*128 + (o-128), hc*128+p]
        # (up layout for chunk 10 is identical to the full-chunk layout.)
        c = NCHUNK - 1
        glast = g[c * CW:(c + 1) * CW].reshape(2, P, HC, P).transpose(0, 2, 3, 1)
        dlast = d[c * CW:(c + 1) * CW].reshape(2, P, HC, P).transpose(0, 2, 3, 1)
        gdlast = np.concatenate([glast, dlast], axis=3)  # [2, HC, P, 256]
        w[c, :, 0:U_OFF] = gdlast.transpose(2, 0, 1, 3).reshape(P, U_OFF)
        in_maps.append({"xt": xt, "eye": eye,
                        "w": np.ascontiguousarray(w)})
    return in_maps


def kernel(x, expert_indices, gate_proj, up_proj, down_proj):
    global LAST_RESULTS
    from concourse.bass_utils import run_bass_kernel_spmd

    nc = _get_compiled()
    in_maps = _pack_inputs(x, gate_proj, up_proj, down_proj)
    res = run_bass_kernel_spmd(nc, in_maps, core_ids=list(range(N_CORES)),
                               trace=TRACE)
    LAST_RESULTS = res

    expert_outs = np.stack([res.results[k]["out"] for k in range(N_CORES)])
    idx = np.asarray(expert_indices).astype(np.int64)  # [T, TOP_K]
    return expert_outs[idx, np.arange(T)[:, None], :].astype(np.float32)
